# revision 5
# baseline (speedup 1.0000x reference)
"""BinSAGE (3-layer bipartite GraphSAGE, mean aggregation) on 8 Trainium2 NeuronCores.

Strategy (graph partition by destination):
- Each layer's target nodes are sharded contiguously across the 8 cores; each core
  holds the edges whose dst falls in its partition.
- Per core, target nodes are degree-sorted and packed into groups of 128 (one SBUF
  partition per node).  Each group's neighbor lists are padded to the group max
  degree (ELL format) with pointers at a known all-zeros row, giving a fully
  regular gather + segment-mean:
    one indirect DMA gathers [128, (1+D)*C] features (slot 0 = the root/target
    node, slots 1..D = neighbors), a strided VectorE reduce sums the neighbor
    slots, a per-partition scale by 1/max(deg,1) forms the mean.
- The SAGE transform runs on the PE: transpose(mean), transpose(root) via
  identity matmuls, then out = meanT^T @ Wl + rootT^T @ Wr + mask^T @ b
  accumulated in PSUM (the mask zeroes rows of padding nodes so a padding row
  of the layer output doubles as the next layer's zero row).
- Layer 0 gathers from a per-core compacted copy of x (only rows referenced by
  that core's edges).  Between layers, slices are AllGather'd into a shared
  full table that the next layer's gathers (and its root loads) index into.
- Final log_softmax computed on-chip; host just concatenates + un-permutes rows.
"""

import numpy as np

import concourse.bass as bass
import concourse.bacc as bacc
import concourse.mybir as mybir
import concourse.tile as tile
from concourse.masks import make_identity

NC = 8
P = 128
F32 = mybir.dt.float32
BF16 = mybir.dt.bfloat16
I32 = mybir.dt.int32

# SBUF budget per partition for one gather chunk (bytes); chunks are ELL groups
# batched into a single indirect DMA to amortize SWDGE fixed overhead.
CHUNK_BYTES = 16384


# --------------------------------------------------------------------------- #
# host-side planning
# --------------------------------------------------------------------------- #

def _layer_plan(src, dst, n_tgt, sort=True):
    """Partition edges by dst; degree-sort each core's targets; shared schedule."""
    s = n_tgt // NC
    assert s * NC == n_tgt
    G = -(-s // P)
    SP = G * P
    per_core = []
    for k in range(NC):
        lo = k * s
        m = (dst >= lo) & (dst < lo + s)
        ds = (dst[m] - lo).astype(np.int64)
        ss = src[m].astype(np.int64)
        deg = np.bincount(ds, minlength=s).astype(np.int64)
        order = (np.argsort(-deg, kind="stable") if sort
                 else np.arange(s, dtype=np.int64))
        e_order = np.argsort(ds, kind="stable")
        csr_src = ss[e_order]
        starts = np.zeros(s + 1, np.int64)
        np.cumsum(deg, out=starts[1:])
        per_core.append((deg, order, csr_src, starts))
    D = []
    for g in range(G):
        i = g * P
        d = max(int(pc[0][pc[1][i]]) for pc in per_core) if i < s else 0
        D.append(d)
    # orig target id -> padded global row of this layer's output table
    row_of_tgt = np.empty(n_tgt, np.int64)
    for k in range(NC):
        order = per_core[k][1]
        row_of_tgt[k * s + order] = k * SP + np.arange(s)
    return dict(s=s, G=G, SP=SP, D=D, per_core=per_core, row_of_tgt=row_of_tgt)


def _core_tables(plan, k, src_row_map, zero_row, fuse_root=True):
    """Build core k's ELL matrix (slot 0 of each group = root row when
    fuse_root, else roots are returned separately), plus rcp [P, G],
    msk [G*P] and the group slot offsets."""
    s, G, D = plan["s"], plan["G"], plan["D"]
    deg, order, csr_src, starts = plan["per_core"][k]
    csr_rows = src_row_map[csr_src] if len(csr_src) else csr_src
    rs = 1 if fuse_root else 0
    tot_slots = rs * G + sum(D)
    ell = np.full((P, tot_slots), zero_row, np.int64)
    roots = np.full(G * P, zero_row, np.int64)
    rcp = np.zeros((P, G), np.float32)
    msk = np.zeros(G * P, np.float32)
    offs = []
    off = 0
    for g in range(G):
        Dg = D[g]
        offs.append(off)
        i0 = g * P
        n = min(P, s - i0)
        nodes = order[i0:i0 + n]
        dg = deg[nodes]
        if fuse_root:
            ell[:n, off] = src_row_map[nodes + k * s]
        roots[g * P:g * P + n] = src_row_map[nodes + k * s]
        if Dg > 0:
            j = np.arange(Dg)[None, :]
            pos = starts[nodes][:, None] + j
            valid = j < dg[:, None]
            pos = np.where(valid, pos, 0)
            tab = np.where(valid, csr_rows[pos], zero_row)
            ell[:n, off + rs:off + rs + Dg] = tab
        rcp[:n, g] = 1.0 / np.maximum(dg, 1)
        msk[g * P:g * P + n] = 1.0
        off += rs + Dg
    return ell, rcp, msk, offs, roots


def _stream_tables(p0, x16):
    """Layer 0 as dense halo streams: per core, each 128-target group's edge
    source rows are written contiguously (dst-sorted, partition-major) so the
    device reads them with one large dense DMA; the edge structure is applied
    on-chip by one-hot matmuls built from the dstrel table."""
    s, G = p0["s"], p0["G"]
    in_c = x16.shape[1]
    # common SPMD schedule: blocks per group = max over cores
    eg = np.zeros((NC, G), np.int64)
    for k in range(NC):
        deg, _, _, starts = p0["per_core"][k]
        for g in range(G):
            eg[k, g] = starts[min((g + 1) * P, s)] - starts[g * P]
    run = -(-eg.max(axis=0) // P)  # [G]
    run = np.maximum(run, 1)
    nboff = np.zeros(G + 1, np.int64)
    np.cumsum(run, out=nboff[1:])
    NB0 = int(nboff[-1])
    xs_list, dr_list, rcp_list, msk_list, xr_list = [], [], [], [], []
    for k in range(NC):
        deg, _, csr_src, starts = p0["per_core"][k]
        xs = np.zeros((NB0 * P, in_c), x16.dtype)
        dr = np.full((P, NB0), 200, np.int32)
        for g in range(G):
            e0, e1 = starts[g * P], starts[min((g + 1) * P, s)]
            n = e1 - e0
            if n == 0:
                continue
            off = int(nboff[g])
            xs[off * P:off * P + n] = x16[csr_src[e0:e1]]
            pos = np.arange(n)
            # dstrel within group for each edge (dst-sorted run lengths)
            drel = np.repeat(
                np.arange(min(P, s - g * P)),
                deg[g * P:min((g + 1) * P, s)],
            ).astype(np.int32)
            dr[pos // run[g], off + pos % run[g]] = drel
        rcp = np.ones((P, G), np.float32)
        msk = np.zeros((1, G * P), np.float32)
        nreal_tail = s - (G - 1) * P
        dg = np.concatenate([deg, np.zeros(G * P - s, np.int64)])
        rcp[:, :] = (1.0 / np.maximum(dg.reshape(G, P), 1)).T
        msk[0, :s] = 1.0
        xr = np.zeros((G * P, in_c), x16.dtype)
        lo = k * s
        xr[:s] = x16[lo:lo + s]
        xs_list.append(xs)
        dr_list.append(dr)
        rcp_list.append(rcp)
        msk_list.append(msk.astype(x16.dtype))
        xr_list.append(xr)
    return dict(run=[int(r) for r in run], nboff=[int(o) for o in nboff],
                NB0=NB0, xs=xs_list, dr=dr_list, rcp=rcp_list,
                msk=msk_list, xr=xr_list)


def _plan_all(x, src0, dst0, src1, dst1, src2, dst2, n1, n2, n3):
    import ml_dtypes
    n0, in_c = x.shape
    p0 = _layer_plan(src0, dst0, n1, sort=False)
    p1 = _layer_plan(src1, dst1, n2)
    p2 = _layer_plan(src2, dst2, n3)

    x16 = x.astype(ml_dtypes.bfloat16)
    st0 = _stream_tables(p0, x16)

    # layers 1 & 2 gather from the gathered padded tables
    assert p0["s"] < p0["SP"] and p1["s"] < p1["SP"], (
        "need a padding row to serve as the zero row"
    )
    zr1 = p0["s"]  # first padding row of core 0's slice (output rows are masked to 0)
    zr2 = p1["s"]
    raw1 = [_core_tables(p1, k, p0["row_of_tgt"], zr1) for k in range(NC)]
    raw2 = [_core_tables(p2, k, p1["row_of_tgt"], zr2) for k in range(NC)]

    def pack(raws, ells=None):
        ells = ells if ells is not None else [r[0] for r in raws]
        return dict(
            ell=[e.astype(np.int32) for e in ells],
            rcp=[r[1] for r in raws],
            msk=[r[2] for r in raws],
            offs=raws[0][3],
        )

    return dict(
        p0=p0, p1=p1, p2=p2, st0=st0,
        t1=pack(raw1), t2=pack(raw2),
    )


# --------------------------------------------------------------------------- #
# device kernel
# --------------------------------------------------------------------------- #

def _chunk_groups(D, cin, rs):
    """Batch consecutive groups into gather chunks within the SBUF budget."""
    chunks, cur, slots = [], [], 0
    cap = max(1, CHUNK_BYTES // (cin * 4))
    for g, d in enumerate(D):
        if cur and slots + d + rs > cap:
            chunks.append(cur)
            cur, slots = [], 0
        cur.append(g)
        slots += d + rs
    if cur:
        chunks.append(cur)
    return chunks


def _emit_layer(nc, tc, lay, feat_ap, out_slice, wl_tiles, wr_tiles, b_tile,
                cin, cout, log_softmax=False, out_ext=None):
    """Emit one SAGE layer inside its own tile pools (freed at layer end)."""
    with (
        tc.tile_pool(name=f"lay{lay['i']}_sbuf", bufs=1) as sbuf,
        tc.tile_pool(name=f"lay{lay['i']}_psum", bufs=1, space="PSUM") as psum,
    ):
        _emit_layer_inner(nc, tc, sbuf, psum, lay, feat_ap, out_slice, wl_tiles,
                          wr_tiles, b_tile, cin, cout, log_softmax, out_ext)


def _emit_layer_inner(nc, tc, sbuf, psum, lay, feat_ap, out_slice, wl_tiles,
                      wr_tiles, b_tile, cin, cout, log_softmax, out_ext):
    G, D, offs = lay["G"], lay["D"], lay["offs"]
    rb = lay.get("rb")          # root-base row in the feature table (layer 0)
    rs = 0 if rb is not None else 1
    ident = lay["ident"]
    nt = -(-cin // P)  # transpose chunks along the feature dim

    slots_tot = rs * G + sum(D)
    ell_sb = sbuf.tile([P, slots_tot], I32, name=f"ell_sb_{lay['i']}")
    nc.sync.dma_start(out=ell_sb[:], in_=lay["ell"][:, :])
    # rcp is consumed by TensorScalarPtr, whose ISA struct has a single sync-wait
    # slot; bounce it through a DVE copy so those reads are same-engine ordered.
    rcp_raw = sbuf.tile([P, G], F32, name=f"rcp_raw_{lay['i']}")
    nc.sync.dma_start(out=rcp_raw[:], in_=lay["rcp"][:, :])
    rcp_sb = sbuf.tile([P, G], F32, name=f"rcp_sb_{lay['i']}")
    nc.vector.tensor_copy(rcp_sb[:], rcp_raw[:])
    msk_sb = sbuf.tile([1, G * P], BF16, name=f"msk_sb_{lay['i']}")
    nc.sync.dma_start(out=msk_sb[:], in_=lay["msk"][None, :])

    for ch in _chunk_groups(D, cin, rs):
        c0 = offs[ch[0]]
        c_slots = sum(D[g] + rs for g in ch)
        msg = sbuf.tile([P, max(c_slots, 1) * cin], BF16, tag=f"msg{lay['i']}",
                        bufs=6, name=f"msg_{lay['i']}_{ch[0]}")
        # HW indirect DMA consumes exactly one index per destination partition
        # row, so gather one ELL slot column (128 rows) per instruction.
        for j in range(c_slots):
            g = nc.gpsimd.indirect_dma_start(
                out=msg[:, j * cin:(j + 1) * cin],
                out_offset=None,
                in_=feat_ap,
                in_offset=bass.IndirectOffsetOnAxis(
                    ap=ell_sb[:, c0 + j:c0 + j + 1], axis=0,
                ),
            )
            if j % 2:
                g.ins.queue = "qPoolDynamic1"  # spread issue over both SWDGE queues
        for g in ch:
            Dg = D[g]
            base = (offs[g] - c0) * cin
            if rs:
                root_ap = msg[:, base:base + cin]
            else:
                # roots are contiguous rows [rb + g*P, rb + (g+1)*P) of the table
                root_t = sbuf.tile([P, cin], BF16, tag="root0", bufs=4,
                                   name=f"root_{lay['i']}_{g}")
                nc.sync.dma_start(out=root_t[:],
                                  in_=feat_ap[rb + g * P:rb + (g + 1) * P, :])
                root_ap = root_t[:]
            mean = sbuf.tile([P, cin], BF16, tag=f"mean{lay['i']}", bufs=3,
                             name=f"mean_{lay['i']}_{g}")
            if Dg > 0:
                # in-place pairwise tree-sum over the Dg neighbor slots:
                # contiguous [P, h*cin] adds keep DVE at streaming rate (the
                # strided "p (j c) -> p c j" reduce ran ~6x below peak).
                s0 = base + rs * cin
                d = Dg
                with nc.allow_low_precision(reason="bf16 neighbor sum, deg<=64"):
                    while d > 1:
                        h = d // 2
                        nc.vector.tensor_tensor(
                            out=msg[:, s0:s0 + h * cin],
                            in0=msg[:, s0:s0 + h * cin],
                            in1=msg[:, s0 + (d - h) * cin:s0 + d * cin],
                            op=mybir.AluOpType.add,
                        )
                        d -= h
                nc.vector.tensor_scalar_mul(mean[:], msg[:, s0:s0 + cin],
                                            rcp_sb[:, g:g + 1])
            else:
                nc.vector.memset(mean[:], 0.0)

            h_ps = psum.tile([P, cout], F32, tag="h_ps", bufs=2,
                             name=f"h_ps_{lay['i']}_{g}")
            first = True
            for pth, tin in ((0, mean[:]), (1, root_ap)):
                for t in range(nt):
                    ct = min(P, cin - t * P)
                    tp = psum.tile([ct, P], BF16, tag="tp", bufs=4,
                                   name=f"tp_{lay['i']}_{g}_{pth}_{t}")
                    nc.tensor.transpose(
                        out=tp[:], in_=tin[:, t * P:t * P + ct], identity=ident[:],
                    )
                    tps = sbuf.tile([ct, P], BF16, tag="tps", bufs=4,
                                    name=f"tps_{lay['i']}_{g}_{pth}_{t}")
                    nc.scalar.copy(tps[:], tp[:])
                    w = (wl_tiles if pth == 0 else wr_tiles)[t]
                    nc.tensor.matmul(h_ps[:], lhsT=tps[:], rhs=w[:],
                                     start=first, stop=False)
                    first = False
            nc.tensor.matmul(h_ps[:], lhsT=msk_sb[:, g * P:(g + 1) * P],
                             rhs=b_tile[:], start=False, stop=True)

            o_sb = sbuf.tile([P, cout], F32 if log_softmax else BF16,
                             tag=f"o{lay['i']}", bufs=3,
                             name=f"o_{lay['i']}_{g}")
            if not log_softmax:
                nc.vector.tensor_copy(o_sb[:], h_ps[:])
                nc.sync.dma_start(out=out_slice[g * P:(g + 1) * P, :], in_=o_sb[:])
            else:
                # hop PSUM->SBUF on DVE first: downstream Ptr-variant ops
                # (Exp with AP bias, tensor_scalar) get single-sem waits.
                h_sb = sbuf.tile([P, cout], F32, tag="h_sb", bufs=2,
                                 name=f"h_sb_{g}")
                nc.vector.tensor_copy(h_sb[:], h_ps[:])
                negm = sbuf.tile([P, 1], F32, tag="negm", bufs=2,
                                 name=f"negm_{g}")
                nc.vector.tensor_reduce(out=negm[:], in_=h_sb[:],
                                        axis=mybir.AxisListType.X,
                                        op=mybir.AluOpType.max, negate=True)
                esum = sbuf.tile([P, 1], F32, tag="esum", bufs=2, name=f"esum_{g}")
                etile = sbuf.tile([P, cout], F32, tag="etile", bufs=2,
                                  name=f"etile_{g}")
                nc.scalar.activation(out=etile[:], in_=h_sb[:],
                                     func=mybir.ActivationFunctionType.Exp,
                                     bias=negm[:], scale=1.0, accum_out=esum[:])
                lns_a = sbuf.tile([P, 1], F32, tag="lns_a", bufs=2,
                                  name=f"lns_a_{g}")
                nc.scalar.activation(out=lns_a[:], in_=esum[:],
                                     func=mybir.ActivationFunctionType.Ln)
                nc.vector.tensor_scalar(
                    out=o_sb[:], in0=h_sb[:], scalar1=negm[:], scalar2=lns_a[:],
                    op0=mybir.AluOpType.add, op1=mybir.AluOpType.subtract,
                )
                nc.sync.dma_start(out=out_ext[g * P:(g + 1) * P, :], in_=o_sb[:])


def _emit_layer0(nc, tc, meta, xs, xr, dr, iot, rcp0, msk0, h1_slice,
                 wl_t, wr_t, b_sb, ident_bf, in_c, hid):
    """Dense-stream layer 0: per 128-target group, one big contiguous DMA of
    the group's (dst-sorted) edge source rows, then segment-sum via one-hot
    matmuls (out comes transposed, ready to be the transform's lhsT)."""
    BF16 = mybir.dt.bfloat16
    run, nboff, G = meta["RUN0"], meta["NBOFF0"], meta["G0"]
    NB0 = meta["NB0"]
    with (
        tc.tile_pool(name="l0_sbuf", bufs=1) as sbuf,
        tc.tile_pool(name="l0_psum", bufs=1, space="PSUM") as psum,
    ):
        dr_sb = sbuf.tile([P, NB0], BF16, name="dr_sb")
        nc.sync.dma_start(out=dr_sb[:], in_=dr[:, :])
        iot_sb = sbuf.tile([P, max(run) * P], BF16, name="iot_sb")
        nc.sync.dma_start(out=iot_sb[:], in_=iot[:, :])
        rcp_raw = sbuf.tile([P, G], F32, name="rcp_raw0")
        nc.sync.dma_start(out=rcp_raw[:], in_=rcp0[:, :])
        rcp_sb = sbuf.tile([P, G], F32, name="rcp_sb0")
        nc.vector.tensor_copy(rcp_sb[:], rcp_raw[:])
        msk_sb = sbuf.tile([1, G * P], BF16, name="msk_sb0")
        nc.sync.dma_start(out=msk_sb[:], in_=msk0[:, :])

        for g in range(G):
            r, off = run[g], nboff[g]
            msg = sbuf.tile([P, r * in_c], BF16, tag="msg0", bufs=3,
                            name=f"msg0_{g}")
            nc.sync.dma_start(
                out=msg[:],
                in_=xs[off * P:(off + r) * P, :]
                    .rearrange("(p r) c -> p (r c)", p=P),
            )
            mg = sbuf.tile([P, r * P], BF16, tag="mg0", bufs=3,
                           name=f"mg0_{g}")
            nc.vector.tensor_tensor(
                out=mg[:].rearrange("p (r c) -> p r c", c=P),
                in0=dr_sb[:, off:off + r]
                    .rearrange("p (r u) -> p r u", u=1)
                    .to_broadcast([P, r, P]),
                in1=iot_sb[:, :r * P].rearrange("p (r c) -> p r c", c=P),
                op=mybir.AluOpType.is_equal,
            )
            mt_ps = psum.tile([in_c, P], F32, tag="mt0", bufs=2,
                              name=f"mt0_{g}")
            for b in range(r):
                nc.tensor.matmul(mt_ps[:], lhsT=msg[:, b * in_c:(b + 1) * in_c],
                                 rhs=mg[:, b * P:(b + 1) * P],
                                 start=(b == 0), stop=(b == r - 1))
            mt_sb = sbuf.tile([in_c, P], BF16, tag="mts0", bufs=2,
                              name=f"mts0_{g}")
            nc.scalar.copy(mt_sb[:], mt_ps[:])

            root_sb = sbuf.tile([P, in_c], BF16, tag="root0", bufs=3,
                                name=f"root0_{g}")
            nc.sync.dma_start(out=root_sb[:], in_=xr[g * P:(g + 1) * P, :])
            rt_ps = psum.tile([in_c, P], BF16, tag="rt0", bufs=2,
                              name=f"rt0_{g}")
            nc.tensor.transpose(out=rt_ps[:], in_=root_sb[:],
                                identity=ident_bf[:])
            rt_sb = sbuf.tile([in_c, P], BF16, tag="rts0", bufs=2,
                              name=f"rts0_{g}")
            nc.scalar.copy(rt_sb[:], rt_ps[:])

            hA = psum.tile([P, hid], F32, tag="hA0", bufs=2, name=f"hA0_{g}")
            nc.tensor.matmul(hA[:], lhsT=mt_sb[:], rhs=wl_t[:],
                             start=True, stop=True)
            hB = psum.tile([P, hid], F32, tag="hB0", bufs=2, name=f"hB0_{g}")
            nc.tensor.matmul(hB[:], lhsT=rt_sb[:], rhs=wr_t[:],
                             start=True, stop=False)
            nc.tensor.matmul(hB[:], lhsT=msk_sb[:, g * P:(g + 1) * P],
                             rhs=b_sb[:], start=False, stop=True)
            h_sb = sbuf.tile([P, hid], BF16, tag="h0", bufs=3, name=f"h0_{g}")
            nc.vector.tensor_scalar_mul(h_sb[:], hA[:], rcp_sb[:, g:g + 1])
            nc.vector.tensor_add(out=h_sb[:], in0=h_sb[:], in1=hB[:])
            nc.sync.dma_start(out=h1_slice[g * P:(g + 1) * P, :], in_=h_sb[:])


def _build_nc(meta):
    """meta: shapes + degree schedules (identical across cores -> one SPMD program)."""
    BF16 = mybir.dt.bfloat16
    in_c, hid, out_c = meta["in_c"], meta["hid"], meta["out_c"]
    nc = bacc.Bacc("TRN2", target_bir_lowering=False, debug=False,
                   num_devices=NC, num_swdge_queues=2)

    NB0, G0 = meta["NB0"], meta["G0"]
    xs = nc.dram_tensor("xs", [NB0 * P, in_c], BF16, kind="ExternalInput")
    xr0 = nc.dram_tensor("xr0", [G0 * P, in_c], BF16, kind="ExternalInput")
    dr0 = nc.dram_tensor("dr0", [P, NB0], BF16, kind="ExternalInput")
    iot = nc.dram_tensor("iot", [P, max(meta["RUN0"]) * P], BF16,
                         kind="ExternalInput")
    rcp0_t = nc.dram_tensor("rcp0", [P, G0], F32, kind="ExternalInput")
    msk0_t = nc.dram_tensor("msk0", [1, G0 * P], BF16, kind="ExternalInput")
    dram_in = {}
    for i, (g, d) in enumerate(((meta["G1"], meta["D1"]),
                               (meta["G2"], meta["D2"])), start=1):
        slots = g + sum(d)  # roots fused as slot 0
        dram_in[f"ell{i}"] = nc.dram_tensor(f"ell{i}", [P, slots], I32,
                                            kind="ExternalInput")
        dram_in[f"rcp{i}"] = nc.dram_tensor(f"rcp{i}", [P, g], F32,
                                            kind="ExternalInput")
        dram_in[f"msk{i}"] = nc.dram_tensor(f"msk{i}", [g * P], BF16,
                                            kind="ExternalInput")
    wl0 = nc.dram_tensor("wl0", [in_c, hid], BF16, kind="ExternalInput")
    wr0 = nc.dram_tensor("wr0", [in_c, hid], BF16, kind="ExternalInput")
    b0 = nc.dram_tensor("b0", [hid], BF16, kind="ExternalInput")
    wl1 = nc.dram_tensor("wl1", [hid, hid], BF16, kind="ExternalInput")
    wr1 = nc.dram_tensor("wr1", [hid, hid], BF16, kind="ExternalInput")
    b1 = nc.dram_tensor("b1", [hid], BF16, kind="ExternalInput")
    wl2 = nc.dram_tensor("wl2", [hid, out_c], BF16, kind="ExternalInput")
    wr2 = nc.dram_tensor("wr2", [hid, out_c], BF16, kind="ExternalInput")
    b2 = nc.dram_tensor("b2", [out_c], BF16, kind="ExternalInput")
    out = nc.dram_tensor("out", [meta["G2"] * P, out_c], F32, kind="ExternalOutput")

    with tile.TileContext(nc) as tc:
        with (
            tc.tile_pool(name="const", bufs=1) as const,
            tc.tile_pool(name="dram", bufs=1, space="DRAM") as dram,
        ):
            ident = const.tile([P, P], F32)
            make_identity(nc, ident[:])
            ident_bf = const.tile([P, P], BF16)
            make_identity(nc, ident_bf[:])

            def load_w(t, rows, cols, dt=F32):
                nt = -(-rows // P)
                tiles = []
                for i in range(nt):
                    ct = min(P, rows - i * P)
                    w_sb = const.tile([ct, cols], dt, name=f"w_{t.name}_{i}")
                    nc.sync.dma_start(out=w_sb[:], in_=t[i * P:i * P + ct, :])
                    tiles.append(w_sb)
                return tiles

            wl0_t = load_w(wl0, in_c, hid, BF16)[0]
            wr0_t = load_w(wr0, in_c, hid, BF16)[0]
            wl1_t, wr1_t = load_w(wl1, hid, hid, BF16), load_w(wr1, hid, hid, BF16)
            wl2_t, wr2_t = load_w(wl2, hid, out_c, BF16), load_w(wr2, hid, out_c, BF16)
            b0_sb = const.tile([1, hid], BF16)
            nc.sync.dma_start(out=b0_sb[:], in_=b0[None, :])
            b1_sb = const.tile([1, hid], BF16)
            nc.sync.dma_start(out=b1_sb[:], in_=b1[None, :])
            b2_sb = const.tile([1, out_c], BF16)
            nc.sync.dma_start(out=b2_sb[:], in_=b2[None, :])

            h1_slice = dram.tile([meta["G0"] * P, hid], BF16)
            h1_full = dram.tile([NC * meta["G0"] * P, hid], BF16,
                                addr_space="Shared")
            h2_slice = dram.tile([meta["G1"] * P, hid], BF16)
            h2_full = dram.tile([NC * meta["G1"] * P, hid], BF16,
                                addr_space="Shared")

            _emit_layer0(nc, tc, meta, xs, xr0, dr0, iot, rcp0_t, msk0_t,
                         h1_slice, wl0_t, wr0_t, b0_sb, ident_bf, in_c, hid)
            nc.gpsimd.collective_compute(
                "AllGather", mybir.AluOpType.bypass,
                replica_groups=[list(range(NC))],
                ins=[h1_slice[:]], outs=[h1_full[:]],
            )

            lay1 = dict(i=1, G=meta["G1"], D=meta["D1"], offs=meta["OFF1"],
                        ident=ident_bf, ell=dram_in["ell1"], rcp=dram_in["rcp1"],
                        msk=dram_in["msk1"])
            _emit_layer(nc, tc, lay1, h1_full[:], h2_slice, wl1_t, wr1_t,
                        b1_sb, hid, hid)
            nc.gpsimd.collective_compute(
                "AllGather", mybir.AluOpType.bypass,
                replica_groups=[list(range(NC))],
                ins=[h2_slice[:]], outs=[h2_full[:]],
            )

            lay2 = dict(i=2, G=meta["G2"], D=meta["D2"], offs=meta["OFF2"],
                        ident=ident_bf, ell=dram_in["ell2"], rcp=dram_in["rcp2"],
                        msk=dram_in["msk2"])
            _emit_layer(nc, tc, lay2, h2_full[:], None, wl2_t, wr2_t,
                        b2_sb, hid, out_c, log_softmax=True, out_ext=out)
    nc.finalize()
    return nc


# --------------------------------------------------------------------------- #
# entry point
# --------------------------------------------------------------------------- #

def _prepare(x, src0, dst0, src1, dst1, src2, dst2, n1, n2, n3,
             Wl0, Wr0, b0, Wl1, Wr1, b1, Wl2, Wr2, b2):
    import ml_dtypes
    BF = ml_dtypes.bfloat16
    x = np.asarray(x, np.float32)
    plan = _plan_all(x, np.asarray(src0), np.asarray(dst0), np.asarray(src1),
                     np.asarray(dst1), np.asarray(src2), np.asarray(dst2),
                     int(n1), int(n2), int(n3))
    p0, p1, p2 = plan["p0"], plan["p1"], plan["p2"]
    st0 = plan["st0"]
    meta = dict(
        in_c=x.shape[1], hid=Wl0.shape[1], out_c=Wl2.shape[1],
        G0=p0["G"], NB0=st0["NB0"], RUN0=st0["run"], NBOFF0=st0["nboff"],
        G1=p1["G"], D1=p1["D"], OFF1=plan["t1"]["offs"],
        G2=p2["G"], D2=p2["D"], OFF2=plan["t2"]["offs"],
    )
    iot = np.tile(np.arange(P, dtype=np.float32), (P, max(st0["run"]))).astype(BF)
    in_maps = []
    for k in range(NC):
        m = dict(
            xs=st0["xs"][k], xr0=st0["xr"][k],
            dr0=np.ascontiguousarray(st0["dr"][k]).astype(np.float32).astype(BF),
            iot=iot,
            rcp0=np.ascontiguousarray(st0["rcp"][k]),
            msk0=np.ascontiguousarray(st0["msk"][k]),
        )
        for i, t in enumerate((plan["t1"], plan["t2"]), start=1):
            m[f"ell{i}"] = np.ascontiguousarray(t["ell"][k])
            m[f"rcp{i}"] = np.ascontiguousarray(t["rcp"][k])
            m[f"msk{i}"] = np.ascontiguousarray(t["msk"][k]).astype(BF)
        m.update(
            wl0=np.asarray(Wl0, np.float32).astype(BF),
            wr0=np.asarray(Wr0, np.float32).astype(BF),
            b0=np.asarray(b0, np.float32).astype(BF),
            wl1=np.asarray(Wl1, np.float32).astype(BF),
            wr1=np.asarray(Wr1, np.float32).astype(BF),
            b1=np.asarray(b1, np.float32).astype(BF),
            wl2=np.asarray(Wl2, np.float32).astype(BF),
            wr2=np.asarray(Wr2, np.float32).astype(BF),
            b2=np.asarray(b2, np.float32).astype(BF),
        )
        in_maps.append(m)
    return plan, meta, in_maps


def _assemble(plan, outs):
    full = np.concatenate(outs, axis=0)  # [NC * G2 * P, out_c] padded rows
    return np.ascontiguousarray(full[plan["p2"]["row_of_tgt"]])


def kernel(**inputs) -> np.ndarray:
    from concourse.bass_utils import run_bass_kernel_spmd

    plan, meta, in_maps = _prepare(**inputs)
    nc = _build_nc(meta)
    res = run_bass_kernel_spmd(nc, in_maps, core_ids=list(range(NC)))
    outs = [res.results[k]["out"] for k in range(NC)]
    return _assemble(plan, outs)



# revision 6
# speedup vs baseline: 1.0303x; 1.0303x over previous
"""BinSAGE v2 (3-layer bipartite GraphSAGE, mean aggregation) on 8 TRN2 cores.

Sharding:
- Node spaces are interleaved across cores in blocks of 4 ids
  (owner(id) = (id//4) % 8), which makes every layer's root features local.
- Layer 0 is dst-sharded: per core, targets are degree-sorted and packed
  into ELL groups of 128; the host pre-gathers the neighbor feature stream
  (one dense DMA per group), the device does a contiguous DVE tree-add
  segment-sum, then the SAGE transform on the PE.
- Layers 1/2 are src-sharded: each core holds the edges whose SOURCE row
  lives in its local feature table, gathers messages with a few big
  dma_gather instructions (994ns fixed cost amortized over thousands of
  rows), one-hot matmuls accumulate partial target sums in canonical
  (owner-major) order, and a bf16 ReduceScatter sums partials across cores.
  Post-collective, each core fetches its (local) root rows with a
  transposed dma_gather, loads the scattered segment with a DMA transpose,
  and runs the transform + (for the last layer) log_softmax.
"""

import numpy as np

import concourse.bass as bass
import concourse.bacc as bacc
import concourse.mybir as mybir
import concourse.tile as tile
from concourse.masks import make_identity

NC = 8
P = 128
IL = 4  # interleave block (ids i: owner = (i//IL) % NC)
F32 = mybir.dt.float32
BF16 = mybir.dt.bfloat16
I16 = mybir.dt.int16

CHUNK_BLOCKS = 28  # gather chunk size (blocks of 128 rows x 512B)
WIN = 4  # target groups per window (512 targets)

IN_C, HID, OUT_C = 100, 256, 47
N0, N1, N2, N3 = 500000, 100000, 25000, 4096
N2V = 25088  # virtual layer-1 target space (ids >= N2 have no edges)
S0, G0, SP0 = 12500, 98, 12544     # layer-0 per-core targets / groups / rows
S1, G1T, SP1 = 3136, 196, 3200     # layer-1 per-core targets / global groups
S2, G2T, SP2 = 512, 32, 512        # layer-2


def _owner(ids):
    return (ids // IL) % NC


def _lpos(ids):
    return (ids // (IL * NC)) * IL + ids % IL


def _tgt_of(k, j):
    """Inverse of (owner, lpos) for target id."""
    return (j // IL) * (IL * NC) + k * IL + j % IL


# --------------------------------------------------------------------------- #
# host planning
# --------------------------------------------------------------------------- #

def _plan_l0(src0, dst0, n1):
    s, G, SP = S0, G0, SP0
    deg = np.bincount(dst0, minlength=n1)
    eo = _owner(dst0)
    ids = np.arange(n1)
    own_t = _owner(ids)
    per_core = []
    Dmax = np.zeros(G, np.int64)
    row1 = np.empty(n1, np.int64)
    slot_of = np.empty(n1, np.int64)
    for k in range(NC):
        tids = ids[own_t == k]                     # ascending, len s
        order = np.argsort(-deg[tids], kind="stable")
        slot_t = tids[order]                       # slot i -> target id
        row1[slot_t] = k * SP + np.arange(s)
        slot_of[slot_t] = np.arange(s)
        em = eo == k
        es, ed = src0[em], dst0[em]
        eslot = slot_of[ed]
        eord = np.argsort(eslot, kind="stable")
        csr_src = es[eord]
        starts = np.zeros(s + 1, np.int64)
        np.cumsum(np.bincount(eslot, minlength=s), out=starts[1:])
        sdeg = deg[slot_t]                         # descending
        gmax = np.array([sdeg[g * P] if g * P < s else 0 for g in range(G)])
        Dmax = np.maximum(Dmax, gmax)
        per_core.append((slot_t, csr_src, starts, sdeg))
    return dict(D=[int(d) for d in Dmax], per_core=per_core, row1=row1)


def _l0_tables(p0, k, x16):
    """Stage core k's ELL stream + roots + rcp + msk."""
    D, (slot_t, csr_src, starts, sdeg) = p0["D"], p0["per_core"][k]
    s, G, SP = S0, G0, SP0
    in_c = x16.shape[1]
    tot = sum(D)
    xs = np.zeros((tot * P, in_c), x16.dtype)
    off = 0
    for g in range(G):
        Dg = D[g]
        if Dg == 0:
            continue
        n = min(P, s - g * P)
        j = np.arange(Dg)[None, :]
        st = starts[g * P:g * P + n][:, None]
        dg = sdeg[g * P:g * P + n][:, None]
        valid = j < dg
        pos = np.where(valid, st + j, 0)
        seg = np.zeros((P, Dg, in_c), x16.dtype)
        rows = csr_src[pos]
        seg[:n][valid] = x16[rows[valid]]
        # [P, in_c, Dg] so the device reduce is innermost-contiguous
        xs[off * P:(off + Dg) * P] = np.ascontiguousarray(
            seg.transpose(0, 2, 1)).reshape(P * Dg, in_c)
        off += Dg
    xr = np.zeros((SP, in_c), x16.dtype)
    xr[:s] = x16[slot_t]
    rcp = np.ones((P, G), np.float32)
    sd = np.concatenate([sdeg, np.zeros(SP - s, np.int64)])
    rcp[:, :] = (1.0 / np.maximum(sd.reshape(G, P), 1)).T
    msk = np.zeros(SP, np.float32)
    msk[:s] = 1.0
    return xs, xr, rcp, msk


def _plan_src_layer(src, dst, n_tgt, row_src, sp_src, zero_row, seg, Gt,
                    rs_chunks=1):
    """Src-sharded layer, window-packed blocks (WIN groups per window),
    chunk-major canonical prow order so the ReduceScatter can be split into
    rs_chunks overlapping collectives.  Blocks are packed densely within a
    window; the common (union) schedule records which groups each block
    touches."""
    tids = np.arange(n_tgt)
    own_t, lp_t = _owner(tids), _lpos(tids)
    piece = seg // rs_chunks
    crows = piece * NC
    prow_t = (lp_t // piece) * crows + own_t * piece + (lp_t % piece)
    eo = _owner(src)
    cnt_global = np.bincount(dst, minlength=n_tgt)
    NW = Gt // WIN
    counts = np.zeros((NC, NW), np.int64)
    core_edges = []
    for k in range(NC):
        m = eo == k
        es, ep = src[m], prow_t[dst[m]]
        o = np.argsort(ep, kind="stable")
        es, ep = es[o], ep[o]
        counts[k] = np.bincount(ep // (WIN * P), minlength=NW)
        core_edges.append((es, ep))
    BW = np.maximum(-(-counts // P), 1).max(axis=0)   # blocks per window
    NB = int(BW.sum())
    woff = np.zeros(NW + 1, np.int64)
    np.cumsum(BW, out=woff[1:])
    # union touch map: touch[b] = set of in-window groups any core hits
    touch = [set() for _ in range(NB)]
    for k in range(NC):
        es, ep = core_edges[k]
        west = np.zeros(NW + 1, np.int64)
        np.cumsum(counts[k], out=west[1:])
        for w in range(NW):
            e0, e1 = west[w], west[w + 1]
            if e1 == e0:
                continue
            gw = (ep[e0:e1] - w * WIN * P) // P
            bl = np.arange(e1 - e0) // P
            for b in range(int(bl[-1]) + 1):
                for g in np.unique(gw[bl == b]):
                    touch[woff[w] + b].add(int(g))
    gmin = np.zeros(NB, np.int64)
    span = np.ones(NB, np.int64)
    for b in range(NB):
        if touch[b]:
            gmin[b] = min(touch[b])
            span[b] = max(touch[b]) - gmin[b] + 1
    # per-window, per-group ordered block lists (window-local block ids)
    sched = []
    for w in range(NW):
        sw = []
        for g in range(WIN):
            sw.append([b for b in range(int(BW[w]))
                       if g in touch[woff[w] + b]])
        sched.append(sw)
    # chunks of whole windows, <= CHUNK_BLOCKS blocks each
    chunks, cur, cb = [], [], 0
    for w in range(NW):
        if cur and cb + BW[w] > CHUNK_BLOCKS:
            chunks.append(cur)
            cur, cb = [], 0
        cur.append(w)
        cb += BW[w]
    if cur:
        chunks.append(cur)
    woh = [int(max(span[woff[w]:woff[w + 1]].max() for w in ws))
           for ws in chunks]
    # per-core tables
    idx_list, dr_list, rcp_list, ridx_list, ep_list = [], [], [], [], []
    for k in range(NC):
        es, ep = core_edges[k]
        idx = np.full(NB * P, zero_row, np.int64)
        drl = np.full(NB * P, 2000.0, np.float32)
        epl = np.full(NB * P, -1, np.int64)
        west = np.zeros(NW + 1, np.int64)
        np.cumsum(counts[k], out=west[1:])
        for w in range(NW):
            e0, e1 = west[w], west[w + 1]
            if e1 == e0:
                continue
            n = e1 - e0
            pos = woff[w] * P + np.arange(n)
            idx[pos] = row_src[es[e0:e1]] - k * sp_src
            dw = ep[e0:e1] - w * WIN * P
            bl = np.arange(n) // P
            drl[pos] = dw - gmin[woff[w] + bl] * P   # shifted in-window drel
            epl[pos] = ep[e0:e1]
        t_loc = _tgt_of(k, np.arange(seg))
        ridx = np.full(-(-seg // P) * P, zero_row, np.int64)
        ridx[:seg] = row_src[t_loc] - k * sp_src
        rcp = np.ones(-(-seg // P) * P, np.float32)
        rcp[:seg] = 1.0 / np.maximum(cnt_global[t_loc], 1)
        idx_list.append(idx)
        dr_list.append(drl)
        rcp_list.append(rcp)
        ridx_list.append(ridx)
        ep_list.append(epl)
    return dict(BW=[int(b) for b in BW], NB=NB, woff=woff, chunks=chunks,
                woh=woh, sched=sched, gmin=gmin, rs_chunks=rs_chunks,
                piece=piece, crows=crows, NW=NW,
                idx=idx_list, dr=dr_list, rcp=rcp_list, ridx=ridx_list,
                eprow=ep_list)


def _pack_idx(idx):
    """idx list (len % 16 == 0) -> [128, n/16] int16 (i at [i%16, i//16])."""
    t = np.asarray(idx, np.int16).reshape(-1, 16).T
    return np.ascontiguousarray(np.tile(t, (8, 1)))


# --------------------------------------------------------------------------- #
# device emitters
# --------------------------------------------------------------------------- #

def _emit_l0(nc, tc, meta, xs0, xr0, rcp0, msk0, h1_slice,
             wl_t, wr_t, b_sb, ident_bf):
    in_c, hid = meta["in_c"], meta["hid"]
    D = meta["D0"]
    with (
        tc.tile_pool(name="l0_sbuf", bufs=1) as sbuf,
        tc.tile_pool(name="l0_psum", bufs=1, space="PSUM") as psum,
    ):
        rcp_raw = sbuf.tile([P, G0], F32, name="rcp_raw0")
        nc.sync.dma_start(out=rcp_raw[:], in_=rcp0[:, :])
        rcp_sb = sbuf.tile([P, G0], F32, name="rcp_sb0")
        nc.vector.tensor_copy(rcp_sb[:], rcp_raw[:])
        msk_sb = sbuf.tile([1, SP0], BF16, name="msk_sb0")
        nc.sync.dma_start(out=msk_sb[:], in_=msk0[None, :])

        off = 0
        for g in range(G0):
            Dg = D[g]
            mean = sbuf.tile([P, in_c], BF16, tag="mean0", bufs=6,
                             name=f"mean0_{g}")
            if Dg > 0:
                # xs0 is staged [P, in_c, Dg] per group: the segment-sum is
                # one contiguous innermost-axis reduce.
                msg = sbuf.tile([P, in_c * Dg], BF16, tag="msg0", bufs=5,
                                name=f"msg0_{g}")
                nc.sync.dma_start(
                    out=msg[:],
                    in_=xs0[off * P:(off + Dg) * P, :]
                        .rearrange("(p j) c -> p (j c)", p=P),
                )
                ssum = sbuf.tile([P, in_c], BF16, tag="ssum0", bufs=6,
                                 name=f"ssum0_{g}")
                with nc.allow_low_precision(reason="bf16 neighbor sum"):
                    nc.vector.tensor_reduce(
                        out=ssum[:],
                        in_=msg[:].rearrange("p (c j) -> p c j", j=Dg),
                        axis=mybir.AxisListType.X,
                        op=mybir.AluOpType.add,
                    )
                nc.vector.tensor_scalar_mul(mean[:], ssum[:],
                                            rcp_sb[:, g:g + 1])
            else:
                nc.vector.memset(mean[:], 0.0)
            root = sbuf.tile([P, in_c], BF16, tag="root0", bufs=6,
                             name=f"root0_{g}")
            nc.sync.dma_start(out=root[:], in_=xr0[g * P:(g + 1) * P, :])

            h_ps = psum.tile([P, hid], F32, tag="hps0", bufs=3,
                             name=f"hps0_{g}")
            first = True
            for tin, w in ((mean, wl_t), (root, wr_t)):
                tp = psum.tile([in_c, P], BF16, tag="tp0", bufs=4,
                               name=f"tp0_{g}_{id(w)}")
                nc.tensor.transpose(out=tp[:], in_=tin[:], identity=ident_bf[:])
                tps = sbuf.tile([in_c, P], BF16, tag="tps0", bufs=6,
                                name=f"tps0_{g}_{id(w)}")
                nc.scalar.copy(tps[:], tp[:])
                nc.tensor.matmul(h_ps[:], lhsT=tps[:], rhs=w[:],
                                 start=first, stop=False)
                first = False
            nc.tensor.matmul(h_ps[:], lhsT=msk_sb[:, g * P:(g + 1) * P],
                             rhs=b_sb[:], start=False, stop=True)
            o = sbuf.tile([P, hid], BF16, tag="o0", bufs=6, name=f"o0_{g}")
            nc.vector.tensor_copy(o[:], h_ps[:])
            nc.scalar.dma_start(out=h1_slice[g * P:(g + 1) * P, :], in_=o[:])
            off += Dg


def _emit_agg(nc, tc, lay, table_ap, partials, cin, rs_emit=None):
    """Src-sharded partial aggregation: chunked dma_gather + one-hot matmuls
    per (window, group); one batched partial write per window."""
    pl = lay["pl"]
    BW, woff, chunks, woh = pl["BW"], pl["woff"], pl["chunks"], pl["woh"]
    sched, gmin, NB = pl["sched"], pl["gmin"], pl["NB"]
    i = lay["i"]
    with (
        tc.tile_pool(name=f"agg{i}_sbuf", bufs=1) as sbuf,
        tc.tile_pool(name=f"agg{i}_psum", bufs=1, space="PSUM") as psum,
    ):
        idx_sb = sbuf.tile([P, NB * 8], I16, name=f"idx_sb{i}")
        nc.sync.dma_start(out=idx_sb[:], in_=lay["idx_t"][:, :])
        dr_raw = sbuf.tile([P, NB], F32, name=f"dr_raw{i}")
        nc.sync.dma_start(out=dr_raw[:], in_=lay["dr_t"][:, :])
        dr_sb = sbuf.tile([P, NB], F32, name=f"dr_sb{i}")
        nc.vector.tensor_copy(dr_sb[:], dr_raw[:])
        iot_sb = sbuf.tile([P, WIN * P], F32, name=f"iot_sb{i}")
        nc.sync.dma_start(out=iot_sb[:], in_=lay["iot_t"][:, :])

        nwpc = pl["crows"] // (WIN * P)   # windows per RS chunk
        for ci, ws in enumerate(chunks):
            b0 = int(woff[ws[0]])
            nb = sum(BW[w] for w in ws)
            wo = woh[ci] * P
            msg = sbuf.tile([P, nb * cin], BF16, tag=f"msg{i}", bufs=4,
                            name=f"msg{i}_{ci}")
            nc.gpsimd.dma_gather(
                msg[:].rearrange("p (b c) -> p b c", c=cin),
                table_ap,
                idx_sb[:, b0 * 8:(b0 + nb) * 8],
                nb * P, nb * P, cin, elem_step=cin, single_packet=False,
                queue_num=ci % 2,
            )
            oh = sbuf.tile([P, nb * wo], BF16, tag=f"oh{i}", bufs=4,
                           name=f"oh{i}_{ci}")
            nc.vector.tensor_tensor(
                out=oh[:].rearrange("p (r c) -> p r c", c=wo),
                in0=dr_sb[:, b0:b0 + nb]
                    .rearrange("p (r u) -> p r u", u=1)
                    .to_broadcast([P, nb, wo]),
                in1=iot_sb[:, :wo].rearrange("p (u c) -> p u c", u=1)
                    .to_broadcast([P, nb, wo]),
                op=mybir.AluOpType.is_equal,
            )
            bb = 0
            for w in ws:
                po = sbuf.tile([P, WIN * cin], BF16, tag=f"po{i}", bufs=6,
                               name=f"po{i}_{w}")
                for g in range(WIN):
                    blocks = sched[w][g]
                    if not blocks:
                        nc.vector.memset(po[:, g * cin:(g + 1) * cin], 0.0)
                        continue
                    h_ps = psum.tile([P, cin], F32, tag=f"hps{i}", bufs=8,
                                     name=f"hps{i}_{w}_{g}")
                    for j, b in enumerate(blocks):
                        gb = woff[w] + b        # global block id
                        col = (g - int(gmin[gb])) * P
                        nc.tensor.matmul(
                            h_ps[:],
                            lhsT=oh[:, (bb + b) * wo + col:
                                    (bb + b) * wo + col + P],
                            rhs=msg[:, (bb + b) * cin:(bb + b + 1) * cin],
                            start=(j == 0), stop=(j == len(blocks) - 1))
                    with nc.allow_low_precision(reason="bf16 partials"):
                        nc.scalar.copy(po[:, g * cin:(g + 1) * cin], h_ps[:])
                part = partials[w // nwpc]
                r0 = (w % nwpc) * WIN * P
                nc.scalar.dma_start(
                    out=part[r0:r0 + WIN * P, :]
                        .rearrange("(b p) c -> p b c", p=P),
                    in_=po[:].rearrange("p (b c) -> p b c", c=cin),
                )
                if rs_emit is not None and (w + 1) % nwpc == 0:
                    rs_emit(w // nwpc)
                bb += BW[w]


def _emit_post(nc, tc, lay, seg_t, table_ap, out_slice, wl_tiles, wr_tiles,
               b_row, cin, cout, seg, sp, log_softmax=False, out_ext=None):
    """Post-RS: transposed loads + root gather + transform (+ log_softmax)."""
    i = lay["i"]
    nt = cin // P  # 2
    SPr = -(-seg // P) * P  # root-gather rows (3200 / 512)
    with (
        tc.tile_pool(name=f"post{i}_sbuf", bufs=1) as sbuf,
        tc.tile_pool(name=f"post{i}_psum", bufs=1, space="PSUM") as psum,
    ):
        ng = SPr // P
        ridx_sb = sbuf.tile([P, SPr // 16], I16, name=f"ridx_sb{i}")
        nc.sync.dma_start(out=ridx_sb[:], in_=lay["ridx_t"][:, :])
        rcp_raw = sbuf.tile([P, ng], F32, name=f"rcp_raw{i}")
        nc.sync.dma_start(out=rcp_raw[:], in_=lay["rcp_t"][:, :])
        rcp_sb = sbuf.tile([P, ng], F32, name=f"rcp_sb{i}")
        nc.vector.tensor_copy(rcp_sb[:], rcp_raw[:])
        ones_sb = sbuf.tile([1, P], BF16, name=f"ones{i}")
        nc.vector.memset(ones_sb[:], 1.0)

        st = []
        for c in range(nt):
            t = sbuf.tile([P, SPr], BF16, name=f"st{i}_{c}")
            nc.sync.dma_start(out=t[:], in_=seg_t[0:SPr, c * P:(c + 1) * P],
                              transpose=True)
            st.append(t)
        rt = sbuf.tile([P, nt * SPr], BF16, name=f"rt{i}")
        nc.gpsimd.dma_gather(
            rt[:].rearrange("p (e n) -> p e n", n=SPr),
            table_ap, ridx_sb[:, :],
            SPr, SPr, cin, elem_step=cin, transpose=True, single_packet=False,
        )

        ng = -(-seg // P)
        for g in range(ng):
            gsz = min(P, seg - g * P)
            hA = psum.tile([P, cout], F32, tag=f"hA_p{i}", bufs=4,
                           name=f"hA_p{i}_{g}")
            hB = psum.tile([P, cout], F32, tag=f"hB_p{i}", bufs=4,
                           name=f"hB_p{i}_{g}")
            for c in range(nt):
                nc.tensor.matmul(hA[:], lhsT=st[c][:, g * P:(g + 1) * P],
                                 rhs=wl_tiles[c][:], start=(c == 0),
                                 stop=(c == nt - 1))
                nc.tensor.matmul(hB[:],
                                 lhsT=rt[:, c * SPr + g * P:c * SPr + (g + 1) * P],
                                 rhs=wr_tiles[c][:], start=(c == 0),
                                 stop=False)
            nc.tensor.matmul(hB[:], lhsT=ones_sb[:], rhs=b_row[:],
                             start=False, stop=True)
            if not log_softmax:
                o = sbuf.tile([P, cout], BF16, tag=f"o_p{i}", bufs=6,
                              name=f"o_p{i}_{g}")
                with nc.allow_low_precision(reason="bf16 mean scale + add"):
                    nc.vector.tensor_scalar_mul(o[:], hA[:],
                                                rcp_sb[:, g:g + 1])
                    nc.vector.tensor_tensor(out=o[:], in0=o[:], in1=hB[:],
                                            op=mybir.AluOpType.add)
                nc.sync.dma_start(out=out_slice[g * P:g * P + gsz, :],
                                  in_=o[:gsz, :])
            else:
                h_sb = sbuf.tile([P, cout], F32, tag="h_sb", bufs=2,
                                 name=f"h_sb{g}")
                nc.vector.tensor_scalar_mul(h_sb[:], hA[:], rcp_sb[:, g:g + 1])
                nc.vector.tensor_tensor(out=h_sb[:], in0=h_sb[:], in1=hB[:],
                                        op=mybir.AluOpType.add)
                negm = sbuf.tile([P, 1], F32, tag="negm", bufs=2,
                                 name=f"negm{g}")
                nc.vector.tensor_reduce(out=negm[:], in_=h_sb[:],
                                        axis=mybir.AxisListType.X,
                                        op=mybir.AluOpType.max, negate=True)
                esum = sbuf.tile([P, 1], F32, tag="esum", bufs=2,
                                 name=f"esum{g}")
                etile = sbuf.tile([P, cout], F32, tag="etile", bufs=2,
                                  name=f"etile{g}")
                nc.scalar.activation(out=etile[:], in_=h_sb[:],
                                     func=mybir.ActivationFunctionType.Exp,
                                     bias=negm[:], scale=1.0, accum_out=esum[:])
                lns = sbuf.tile([P, 1], F32, tag="lns", bufs=2,
                                name=f"lns{g}")
                nc.scalar.activation(out=lns[:], in_=esum[:],
                                     func=mybir.ActivationFunctionType.Ln)
                o = sbuf.tile([P, cout], F32, tag="o_ls", bufs=2,
                              name=f"o_ls{g}")
                nc.vector.tensor_scalar(
                    out=o[:], in0=h_sb[:], scalar1=negm[:], scalar2=lns[:],
                    op0=mybir.AluOpType.add, op1=mybir.AluOpType.subtract,
                )
                nc.sync.dma_start(out=out_ext[g * P:g * P + gsz, :],
                                  in_=o[:gsz, :])


# --------------------------------------------------------------------------- #
# program builder
# --------------------------------------------------------------------------- #

def _build_nc(meta):
    in_c, hid, out_c = meta["in_c"], meta["hid"], meta["out_c"]
    nc = bacc.Bacc("TRN2", target_bir_lowering=False, debug=False,
                   num_devices=NC, num_swdge_queues=2)

    TD0 = sum(meta["D0"])
    xs0 = nc.dram_tensor("xs0", [TD0 * P, in_c], BF16, kind="ExternalInput")
    xr0 = nc.dram_tensor("xr0", [SP0, in_c], BF16, kind="ExternalInput")
    rcp0 = nc.dram_tensor("rcp0", [P, G0], F32, kind="ExternalInput")
    msk0 = nc.dram_tensor("msk0", [SP0], BF16, kind="ExternalInput")
    iot = nc.dram_tensor("iot", [P, WIN * P], F32, kind="ExternalInput")
    din = {}
    for i, (nb, spr) in enumerate(((meta["NB1"], SP1), (meta["NB2"], SP2)),
                                  start=1):
        din[f"idx{i}"] = nc.dram_tensor(f"idx{i}", [P, nb * 8], I16,
                                        kind="ExternalInput")
        din[f"dr{i}"] = nc.dram_tensor(f"dr{i}", [P, nb], F32,
                                       kind="ExternalInput")
        din[f"ridx{i}"] = nc.dram_tensor(f"ridx{i}", [P, spr // 16], I16,
                                         kind="ExternalInput")
        din[f"rcp{i}"] = nc.dram_tensor(f"rcp{i}", [P, spr // P], F32,
                                        kind="ExternalInput")
    wl0 = nc.dram_tensor("wl0", [in_c, hid], BF16, kind="ExternalInput")
    wr0 = nc.dram_tensor("wr0", [in_c, hid], BF16, kind="ExternalInput")
    b0 = nc.dram_tensor("b0", [hid], BF16, kind="ExternalInput")
    wl1 = nc.dram_tensor("wl1", [hid, hid], BF16, kind="ExternalInput")
    wr1 = nc.dram_tensor("wr1", [hid, hid], BF16, kind="ExternalInput")
    b1 = nc.dram_tensor("b1", [hid], BF16, kind="ExternalInput")
    wl2 = nc.dram_tensor("wl2", [hid, out_c], BF16, kind="ExternalInput")
    wr2 = nc.dram_tensor("wr2", [hid, out_c], BF16, kind="ExternalInput")
    b2 = nc.dram_tensor("b2", [out_c], BF16, kind="ExternalInput")
    out = nc.dram_tensor("out", [S2, out_c], F32, kind="ExternalOutput")

    with tile.TileContext(nc) as tc:
        with (
            tc.tile_pool(name="const", bufs=1) as const,
            tc.tile_pool(name="dram", bufs=1, space="DRAM") as dram,
        ):
            ident_bf = const.tile([P, P], BF16)
            make_identity(nc, ident_bf[:])

            def load_w(t, rows, cols):
                tiles = []
                for i in range(-(-rows // P)):
                    ct = min(P, rows - i * P)
                    w_sb = const.tile([ct, cols], BF16, name=f"w_{t.name}_{i}")
                    nc.sync.dma_start(out=w_sb[:], in_=t[i * P:i * P + ct, :])
                    tiles.append(w_sb)
                return tiles

            wl0_t = load_w(wl0, in_c, hid)[0]
            wr0_t = load_w(wr0, in_c, hid)[0]
            wl1_t, wr1_t = load_w(wl1, hid, hid), load_w(wr1, hid, hid)
            wl2_t, wr2_t = load_w(wl2, hid, out_c), load_w(wr2, hid, out_c)
            b0_sb = const.tile([1, hid], BF16)
            nc.sync.dma_start(out=b0_sb[:], in_=b0[None, :])
            b1_sb = const.tile([1, hid], BF16)
            nc.sync.dma_start(out=b1_sb[:], in_=b1[None, :])
            b2_sb = const.tile([1, out_c], BF16)
            nc.sync.dma_start(out=b2_sb[:], in_=b2[None, :])

            pl1, pl2 = meta["pl1"], meta["pl2"]
            h1_slice = dram.tile([SP0, hid], BF16)
            partials1 = [dram.tile([pl1["crows"], hid], BF16,
                                   name=f"partial1_{c}")
                         for c in range(pl1["rs_chunks"])]
            seg1 = dram.tile([SP1, hid], BF16)
            h2_slice = dram.tile([SP1, hid], BF16)
            partials2 = [dram.tile([pl2["crows"], hid], BF16,
                                   name=f"partial2_{c}")
                         for c in range(pl2["rs_chunks"])]
            seg2 = dram.tile([SP2, hid], BF16)

            # zero the padding rows of h2_slice (layer-2 gather zero rows)
            zpad = const.tile([SP1 - S1, hid], BF16)
            nc.vector.memset(zpad[:], 0.0)
            nc.sync.dma_start(out=h2_slice[S1:SP1, :], in_=zpad[:])

            _emit_l0(nc, tc, meta, xs0, xr0, rcp0, msk0, h1_slice,
                     wl0_t, wr0_t, b0_sb, ident_bf)

            lay1 = dict(i=1, pl=pl1, NB=pl1["NB"],
                        idx_t=din["idx1"], dr_t=din["dr1"], iot_t=iot,
                        ridx_t=din["ridx1"], rcp_t=din["rcp1"])
            def rs1_emit(c):
                nc.gpsimd.collective_compute(
                    "ReduceScatter", mybir.AluOpType.add,
                    replica_groups=[list(range(NC))],
                    ins=[partials1[c][:]],
                    outs=[seg1[c * pl1["piece"]:(c + 1) * pl1["piece"], :]],
                )
            _emit_agg(nc, tc, lay1, h1_slice[:, :], partials1, hid,
                      rs_emit=rs1_emit)
            _emit_post(nc, tc, lay1, seg1, h1_slice[:, :], h2_slice,
                       wl1_t, wr1_t, b1_sb, hid, hid, S1, SP0)

            lay2 = dict(i=2, pl=pl2, NB=pl2["NB"],
                        idx_t=din["idx2"], dr_t=din["dr2"], iot_t=iot,
                        ridx_t=din["ridx2"], rcp_t=din["rcp2"])
            def rs2_emit(c):
                nc.gpsimd.collective_compute(
                    "ReduceScatter", mybir.AluOpType.add,
                    replica_groups=[list(range(NC))],
                    ins=[partials2[c][:]],
                    outs=[seg2[c * pl2["piece"]:(c + 1) * pl2["piece"], :]],
                )
            _emit_agg(nc, tc, lay2, h2_slice[:, :], partials2, hid,
                      rs_emit=rs2_emit)
            _emit_post(nc, tc, lay2, seg2, h2_slice[:, :], None,
                       wl2_t, wr2_t, b2_sb, hid, out_c, S2, SP1,
                       log_softmax=True, out_ext=out)
    nc.finalize()
    return nc


# --------------------------------------------------------------------------- #
# entry point
# --------------------------------------------------------------------------- #

def _prepare(x, src0, dst0, src1, dst1, src2, dst2, n1, n2, n3,
             Wl0, Wr0, b0, Wl1, Wr1, b1, Wl2, Wr2, b2):
    import ml_dtypes
    BF = ml_dtypes.bfloat16
    x16 = np.asarray(x, np.float32).astype(BF)
    src0, dst0 = np.asarray(src0, np.int64), np.asarray(dst0, np.int64)
    src1, dst1 = np.asarray(src1, np.int64), np.asarray(dst1, np.int64)
    src2, dst2 = np.asarray(src2, np.int64), np.asarray(dst2, np.int64)
    assert (int(n1), int(n2), int(n3)) == (N1, N2, N3)

    p0 = _plan_l0(src0, dst0, N1)
    row2 = _owner(np.arange(N2V)) * SP1 + _lpos(np.arange(N2V))
    pl1 = _plan_src_layer(src1, dst1, N2V, p0["row1"], SP0, S0, S1, G1T,
                          rs_chunks=7)
    pl2 = _plan_src_layer(src2, dst2, N3, row2, SP1, S1, S2, G2T,
                          rs_chunks=1)

    meta = dict(
        in_c=x16.shape[1], hid=Wl0.shape[1], out_c=Wl2.shape[1],
        D0=p0["D"], pl1=pl1, pl2=pl2, NB1=pl1["NB"], NB2=pl2["NB"],
    )
    iot = np.tile(np.arange(WIN * P, dtype=np.float32)[None, :], (P, 1))
    in_maps = []
    for k in range(NC):
        xs, xr, rcp, msk = _l0_tables(p0, k, x16)
        m = dict(
            xs0=xs, xr0=xr, rcp0=np.ascontiguousarray(rcp),
            msk0=msk.astype(BF), iot=iot,
            wl0=np.asarray(Wl0, np.float32).astype(BF),
            wr0=np.asarray(Wr0, np.float32).astype(BF),
            b0=np.asarray(b0, np.float32).astype(BF),
            wl1=np.asarray(Wl1, np.float32).astype(BF),
            wr1=np.asarray(Wr1, np.float32).astype(BF),
            b1=np.asarray(b1, np.float32).astype(BF),
            wl2=np.asarray(Wl2, np.float32).astype(BF),
            wr2=np.asarray(Wr2, np.float32).astype(BF),
            b2=np.asarray(b2, np.float32).astype(BF),
        )
        for i, pl in ((1, pl1), (2, pl2)):
            m[f"idx{i}"] = _pack_idx(pl["idx"][k])
            m[f"dr{i}"] = np.ascontiguousarray(
                pl["dr"][k].reshape(-1, P).T.astype(np.float32))
            m[f"ridx{i}"] = _pack_idx(pl["ridx"][k])
            m[f"rcp{i}"] = np.ascontiguousarray(
                pl["rcp"][k].reshape(-1, P).T)
        in_maps.append(m)
    return (p0, pl1, pl2), meta, in_maps


def _assemble(outs):
    t = np.arange(N3)
    full = np.stack(outs)  # [NC, S2, out_c]
    return np.ascontiguousarray(full[_owner(t), _lpos(t)])


def kernel(**inputs) -> np.ndarray:
    from concourse.bass_utils import run_bass_kernel_spmd

    _, meta, in_maps = _prepare(**inputs)
    nc = _build_nc(meta)
    res = run_bass_kernel_spmd(nc, in_maps, core_ids=list(range(NC)))
    return _assemble([res.results[k]["out"] for k in range(NC)])


# revision 7
# speedup vs baseline: 1.0358x; 1.0053x over previous
"""BinSAGE v2 (3-layer bipartite GraphSAGE, mean aggregation) on 8 TRN2 cores.

Sharding:
- Node spaces are interleaved across cores in blocks of 4 ids
  (owner(id) = (id//4) % 8), which makes every layer's root features local.
- Layer 0 is dst-sharded: per core, targets are degree-sorted and packed
  into ELL groups of 128; the host pre-gathers the neighbor feature stream
  (one dense DMA per group), the device does a contiguous DVE tree-add
  segment-sum, then the SAGE transform on the PE.
- Layers 1/2 are src-sharded: each core holds the edges whose SOURCE row
  lives in its local feature table, gathers messages with a few big
  dma_gather instructions (994ns fixed cost amortized over thousands of
  rows), one-hot matmuls accumulate partial target sums in canonical
  (owner-major) order, and a bf16 ReduceScatter sums partials across cores.
  Post-collective, each core fetches its (local) root rows with a
  transposed dma_gather, loads the scattered segment with a DMA transpose,
  and runs the transform + (for the last layer) log_softmax.
"""

import numpy as np

import concourse.bass as bass
import concourse.bacc as bacc
import concourse.mybir as mybir
import concourse.tile as tile
from concourse.masks import make_identity

NC = 8
P = 128
IL = 4  # interleave block (ids i: owner = (i//IL) % NC)
F32 = mybir.dt.float32
BF16 = mybir.dt.bfloat16
I16 = mybir.dt.int16

CHUNK_BLOCKS = 28  # gather chunk size (blocks of 128 rows x 512B)
WIN = 4  # target groups per window (512 targets)

IN_C, HID, OUT_C = 100, 256, 47
N0, N1, N2, N3 = 500000, 100000, 25000, 4096
N2V = 25088  # virtual layer-1 target space (ids >= N2 have no edges)
S0, G0, SP0 = 12500, 98, 12544     # layer-0 per-core targets / groups / rows
S1, G1T, SP1 = 3136, 196, 3200     # layer-1 per-core targets / global groups
S2, G2T, SP2 = 512, 32, 512        # layer-2


def _owner(ids):
    return (ids // IL) % NC


def _lpos(ids):
    return (ids // (IL * NC)) * IL + ids % IL


def _tgt_of(k, j):
    """Inverse of (owner, lpos) for target id."""
    return (j // IL) * (IL * NC) + k * IL + j % IL


# --------------------------------------------------------------------------- #
# host planning
# --------------------------------------------------------------------------- #

def _plan_l0(src0, dst0, n1):
    s, G, SP = S0, G0, SP0
    deg = np.bincount(dst0, minlength=n1)
    eo = _owner(dst0)
    ids = np.arange(n1)
    own_t = _owner(ids)
    per_core = []
    Dmax = np.zeros(G, np.int64)
    row1 = np.empty(n1, np.int64)
    slot_of = np.empty(n1, np.int64)
    for k in range(NC):
        tids = ids[own_t == k]                     # ascending, len s
        order = np.argsort(-deg[tids], kind="stable")
        slot_t = tids[order]                       # slot i -> target id
        row1[slot_t] = k * SP + np.arange(s)
        slot_of[slot_t] = np.arange(s)
        em = eo == k
        es, ed = src0[em], dst0[em]
        eslot = slot_of[ed]
        eord = np.argsort(eslot, kind="stable")
        csr_src = es[eord]
        starts = np.zeros(s + 1, np.int64)
        np.cumsum(np.bincount(eslot, minlength=s), out=starts[1:])
        sdeg = deg[slot_t]                         # descending
        gmax = np.array([sdeg[g * P] if g * P < s else 0 for g in range(G)])
        Dmax = np.maximum(Dmax, gmax)
        per_core.append((slot_t, csr_src, starts, sdeg))
    return dict(D=[int(d) for d in Dmax], per_core=per_core, row1=row1)


def _l0_tables(p0, k, x16):
    """Stage core k's ELL stream + roots + rcp + msk."""
    D, (slot_t, csr_src, starts, sdeg) = p0["D"], p0["per_core"][k]
    s, G, SP = S0, G0, SP0
    in_c = x16.shape[1]
    tot = sum(D)
    xs = np.zeros((tot * P, in_c), x16.dtype)
    off = 0
    for g in range(G):
        Dg = D[g]
        if Dg == 0:
            continue
        n = min(P, s - g * P)
        j = np.arange(Dg)[None, :]
        st = starts[g * P:g * P + n][:, None]
        dg = sdeg[g * P:g * P + n][:, None]
        valid = j < dg
        pos = np.where(valid, st + j, 0)
        seg = np.zeros((P, Dg, in_c), x16.dtype)
        rows = csr_src[pos]
        seg[:n][valid] = x16[rows[valid]]
        # [P, in_c, Dg] so the device reduce is innermost-contiguous
        xs[off * P:(off + Dg) * P] = np.ascontiguousarray(
            seg.transpose(0, 2, 1)).reshape(P * Dg, in_c)
        off += Dg
    xr = np.zeros((SP, in_c), x16.dtype)
    xr[:s] = x16[slot_t]
    rcp = np.ones((P, G), np.float32)
    sd = np.concatenate([sdeg, np.zeros(SP - s, np.int64)])
    rcp[:, :] = (1.0 / np.maximum(sd.reshape(G, P), 1)).T
    msk = np.zeros(SP, np.float32)
    msk[:s] = 1.0
    return xs, xr, rcp, msk


def _plan_src_layer(src, dst, n_tgt, row_src, sp_src, zero_row, seg, Gt,
                    rs_chunks=1):
    """Src-sharded layer, window-packed blocks (WIN groups per window),
    chunk-major canonical prow order so the ReduceScatter can be split into
    rs_chunks overlapping collectives.  Blocks are packed densely within a
    window; the common (union) schedule records which groups each block
    touches."""
    tids = np.arange(n_tgt)
    own_t, lp_t = _owner(tids), _lpos(tids)
    piece = seg // rs_chunks
    crows = piece * NC
    prow_t = (lp_t // piece) * crows + own_t * piece + (lp_t % piece)
    eo = _owner(src)
    cnt_global = np.bincount(dst, minlength=n_tgt)
    NW = Gt // WIN
    counts = np.zeros((NC, NW), np.int64)
    core_edges = []
    for k in range(NC):
        m = eo == k
        es, ep = src[m], prow_t[dst[m]]
        o = np.argsort(ep, kind="stable")
        es, ep = es[o], ep[o]
        counts[k] = np.bincount(ep // (WIN * P), minlength=NW)
        core_edges.append((es, ep))
    BW = np.maximum(-(-counts // P), 1).max(axis=0)   # blocks per window
    NB = int(BW.sum())
    woff = np.zeros(NW + 1, np.int64)
    np.cumsum(BW, out=woff[1:])
    # union touch map: touch[b] = set of in-window groups any core hits
    touch = [set() for _ in range(NB)]
    for k in range(NC):
        es, ep = core_edges[k]
        west = np.zeros(NW + 1, np.int64)
        np.cumsum(counts[k], out=west[1:])
        for w in range(NW):
            e0, e1 = west[w], west[w + 1]
            if e1 == e0:
                continue
            gw = (ep[e0:e1] - w * WIN * P) // P
            bl = np.arange(e1 - e0) // P
            for b in range(int(bl[-1]) + 1):
                for g in np.unique(gw[bl == b]):
                    touch[woff[w] + b].add(int(g))
    gmin = np.zeros(NB, np.int64)
    span = np.ones(NB, np.int64)
    for b in range(NB):
        if touch[b]:
            gmin[b] = min(touch[b])
            span[b] = max(touch[b]) - gmin[b] + 1
    # per-window, per-group ordered block lists (window-local block ids)
    sched = []
    for w in range(NW):
        sw = []
        for g in range(WIN):
            sw.append([b for b in range(int(BW[w]))
                       if g in touch[woff[w] + b]])
        sched.append(sw)
    # chunks of whole windows, <= CHUNK_BLOCKS blocks each
    chunks, cur, cb = [], [], 0
    for w in range(NW):
        if cur and cb + BW[w] > CHUNK_BLOCKS:
            chunks.append(cur)
            cur, cb = [], 0
        cur.append(w)
        cb += BW[w]
    if cur:
        chunks.append(cur)
    woh = [int(max(span[woff[w]:woff[w + 1]].max() for w in ws))
           for ws in chunks]
    # per-core tables
    idx_list, dr_list, rcp_list, ridx_list, ep_list = [], [], [], [], []
    for k in range(NC):
        es, ep = core_edges[k]
        idx = np.full(NB * P, zero_row, np.int64)
        drl = np.full(NB * P, 2000.0, np.float32)
        epl = np.full(NB * P, -1, np.int64)
        west = np.zeros(NW + 1, np.int64)
        np.cumsum(counts[k], out=west[1:])
        for w in range(NW):
            e0, e1 = west[w], west[w + 1]
            if e1 == e0:
                continue
            n = e1 - e0
            pos = woff[w] * P + np.arange(n)
            idx[pos] = row_src[es[e0:e1]] - k * sp_src
            dw = ep[e0:e1] - w * WIN * P
            bl = np.arange(n) // P
            drl[pos] = dw - gmin[woff[w] + bl] * P   # shifted in-window drel
            epl[pos] = ep[e0:e1]
        t_loc = _tgt_of(k, np.arange(seg))
        ridx = np.full(-(-seg // P) * P, zero_row, np.int64)
        ridx[:seg] = row_src[t_loc] - k * sp_src
        rcp = np.ones(-(-seg // P) * P, np.float32)
        rcp[:seg] = 1.0 / np.maximum(cnt_global[t_loc], 1)
        idx_list.append(idx)
        dr_list.append(drl)
        rcp_list.append(rcp)
        ridx_list.append(ridx)
        ep_list.append(epl)
    return dict(BW=[int(b) for b in BW], NB=NB, woff=woff, chunks=chunks,
                woh=woh, sched=sched, gmin=gmin, rs_chunks=rs_chunks,
                piece=piece, crows=crows, NW=NW,
                idx=idx_list, dr=dr_list, rcp=rcp_list, ridx=ridx_list,
                eprow=ep_list)


def _pack_idx(idx):
    """idx list (len % 16 == 0) -> [128, n/16] int16 (i at [i%16, i//16])."""
    t = np.asarray(idx, np.int16).reshape(-1, 16).T
    return np.ascontiguousarray(np.tile(t, (8, 1)))


# --------------------------------------------------------------------------- #
# device emitters
# --------------------------------------------------------------------------- #

def _emit_l0(nc, tc, meta, xs0, xr0, rcp0, msk0, h1_slice,
             wl_t, wr_t, b_sb, ident_bf):
    in_c, hid = meta["in_c"], meta["hid"]
    D = meta["D0"]
    with (
        tc.tile_pool(name="l0_sbuf", bufs=1) as sbuf,
        tc.tile_pool(name="l0_psum", bufs=1, space="PSUM") as psum,
    ):
        rcp_raw = sbuf.tile([P, G0], F32, name="rcp_raw0")
        nc.sync.dma_start(out=rcp_raw[:], in_=rcp0[:, :])
        rcp_sb = sbuf.tile([P, G0], F32, name="rcp_sb0")
        nc.vector.tensor_copy(rcp_sb[:], rcp_raw[:])
        msk_sb = sbuf.tile([1, SP0], BF16, name="msk_sb0")
        nc.sync.dma_start(out=msk_sb[:], in_=msk0[None, :])

        off = 0
        for g in range(G0):
            Dg = D[g]
            mean = sbuf.tile([P, in_c], BF16, tag="mean0", bufs=6,
                             name=f"mean0_{g}")
            if Dg > 0:
                # xs0 is staged [P, in_c, Dg] per group: the segment-sum is
                # one contiguous innermost-axis reduce.
                msg = sbuf.tile([P, in_c * Dg], BF16, tag="msg0", bufs=5,
                                name=f"msg0_{g}")
                nc.sync.dma_start(
                    out=msg[:],
                    in_=xs0[off * P:(off + Dg) * P, :]
                        .rearrange("(p j) c -> p (j c)", p=P),
                )
                ssum = sbuf.tile([P, in_c], BF16, tag="ssum0", bufs=6,
                                 name=f"ssum0_{g}")
                with nc.allow_low_precision(reason="bf16 neighbor sum"):
                    nc.vector.tensor_reduce(
                        out=ssum[:],
                        in_=msg[:].rearrange("p (c j) -> p c j", j=Dg),
                        axis=mybir.AxisListType.X,
                        op=mybir.AluOpType.add,
                    )
                nc.scalar.activation(out=mean[:], in_=ssum[:],
                                     func=mybir.ActivationFunctionType.Copy,
                                     scale=rcp_sb[:, g:g + 1])
            else:
                nc.vector.memset(mean[:], 0.0)
            root = sbuf.tile([P, in_c], BF16, tag="root0", bufs=6,
                             name=f"root0_{g}")
            nc.sync.dma_start(out=root[:], in_=xr0[g * P:(g + 1) * P, :])

            h_ps = psum.tile([P, hid], F32, tag="hps0", bufs=3,
                             name=f"hps0_{g}")
            first = True
            for tin, w in ((mean, wl_t), (root, wr_t)):
                tp = psum.tile([in_c, P], BF16, tag="tp0", bufs=4,
                               name=f"tp0_{g}_{id(w)}")
                nc.tensor.transpose(out=tp[:], in_=tin[:], identity=ident_bf[:])
                tps = sbuf.tile([in_c, P], BF16, tag="tps0", bufs=6,
                                name=f"tps0_{g}_{id(w)}")
                nc.scalar.copy(tps[:], tp[:])
                nc.tensor.matmul(h_ps[:], lhsT=tps[:], rhs=w[:],
                                 start=first, stop=False)
                first = False
            nc.tensor.matmul(h_ps[:], lhsT=msk_sb[:, g * P:(g + 1) * P],
                             rhs=b_sb[:], start=False, stop=True)
            o = sbuf.tile([P, hid], BF16, tag="o0", bufs=6, name=f"o0_{g}")
            nc.vector.tensor_copy(o[:], h_ps[:])
            nc.scalar.dma_start(out=h1_slice[g * P:(g + 1) * P, :], in_=o[:])
            off += Dg


def _emit_agg(nc, tc, lay, table_ap, partials, cin, rs_emit=None):
    """Src-sharded partial aggregation: chunked dma_gather + one-hot matmuls
    per (window, group); one batched partial write per window."""
    pl = lay["pl"]
    BW, woff, chunks, woh = pl["BW"], pl["woff"], pl["chunks"], pl["woh"]
    sched, gmin, NB = pl["sched"], pl["gmin"], pl["NB"]
    i = lay["i"]
    with (
        tc.tile_pool(name=f"agg{i}_sbuf", bufs=1) as sbuf,
        tc.tile_pool(name=f"agg{i}_psum", bufs=1, space="PSUM") as psum,
    ):
        idx_sb = sbuf.tile([P, NB * 8], I16, name=f"idx_sb{i}")
        nc.sync.dma_start(out=idx_sb[:], in_=lay["idx_t"][:, :])
        dr_raw = sbuf.tile([P, NB], F32, name=f"dr_raw{i}")
        nc.sync.dma_start(out=dr_raw[:], in_=lay["dr_t"][:, :])
        dr_sb = sbuf.tile([P, NB], F32, name=f"dr_sb{i}")
        nc.vector.tensor_copy(dr_sb[:], dr_raw[:])
        iot_sb = sbuf.tile([P, WIN * P], F32, name=f"iot_sb{i}")
        nc.sync.dma_start(out=iot_sb[:], in_=lay["iot_t"][:, :])

        nwpc = pl["crows"] // (WIN * P)   # windows per RS chunk
        for ci, ws in enumerate(chunks):
            b0 = int(woff[ws[0]])
            nb = sum(BW[w] for w in ws)
            wo = woh[ci] * P
            msg = sbuf.tile([P, nb * cin], BF16, tag=f"msg{i}", bufs=4,
                            name=f"msg{i}_{ci}")
            nc.gpsimd.dma_gather(
                msg[:].rearrange("p (b c) -> p b c", c=cin),
                table_ap,
                idx_sb[:, b0 * 8:(b0 + nb) * 8],
                nb * P, nb * P, cin, elem_step=cin, single_packet=False,
                queue_num=ci % 2,
            )
            oh = sbuf.tile([P, nb * wo], BF16, tag=f"oh{i}", bufs=4,
                           name=f"oh{i}_{ci}")
            nc.vector.tensor_tensor(
                out=oh[:].rearrange("p (r c) -> p r c", c=wo),
                in0=dr_sb[:, b0:b0 + nb]
                    .rearrange("p (r u) -> p r u", u=1)
                    .to_broadcast([P, nb, wo]),
                in1=iot_sb[:, :wo].rearrange("p (u c) -> p u c", u=1)
                    .to_broadcast([P, nb, wo]),
                op=mybir.AluOpType.is_equal,
            )
            bb = 0
            for w in ws:
                po = sbuf.tile([P, WIN * cin], BF16, tag=f"po{i}", bufs=6,
                               name=f"po{i}_{w}")
                for g in range(WIN):
                    blocks = sched[w][g]
                    if not blocks:
                        nc.vector.memset(po[:, g * cin:(g + 1) * cin], 0.0)
                        continue
                    h_ps = psum.tile([P, cin], F32, tag=f"hps{i}", bufs=8,
                                     name=f"hps{i}_{w}_{g}")
                    for j, b in enumerate(blocks):
                        gb = woff[w] + b        # global block id
                        col = (g - int(gmin[gb])) * P
                        nc.tensor.matmul(
                            h_ps[:],
                            lhsT=oh[:, (bb + b) * wo + col:
                                    (bb + b) * wo + col + P],
                            rhs=msg[:, (bb + b) * cin:(bb + b + 1) * cin],
                            start=(j == 0), stop=(j == len(blocks) - 1))
                    with nc.allow_low_precision(reason="bf16 partials"):
                        nc.scalar.copy(po[:, g * cin:(g + 1) * cin], h_ps[:])
                part = partials[w // nwpc]
                r0 = (w % nwpc) * WIN * P
                nc.scalar.dma_start(
                    out=part[r0:r0 + WIN * P, :]
                        .rearrange("(b p) c -> p b c", p=P),
                    in_=po[:].rearrange("p (b c) -> p b c", c=cin),
                )
                if rs_emit is not None and (w + 1) % nwpc == 0:
                    rs_emit(w // nwpc)
                bb += BW[w]


def _emit_post(nc, tc, lay, seg_t, table_ap, out_slice, wl_tiles, wr_tiles,
               b_row, cin, cout, seg, sp, log_softmax=False, out_ext=None):
    """Post-RS: transposed loads + root gather + transform (+ log_softmax)."""
    i = lay["i"]
    nt = cin // P  # 2
    SPr = -(-seg // P) * P  # root-gather rows (3200 / 512)
    with (
        tc.tile_pool(name=f"post{i}_sbuf", bufs=1) as sbuf,
        tc.tile_pool(name=f"post{i}_psum", bufs=1, space="PSUM") as psum,
    ):
        ng = SPr // P
        ridx_sb = sbuf.tile([P, SPr // 16], I16, name=f"ridx_sb{i}")
        nc.sync.dma_start(out=ridx_sb[:], in_=lay["ridx_t"][:, :])
        rcp_raw = sbuf.tile([P, ng], F32, name=f"rcp_raw{i}")
        nc.sync.dma_start(out=rcp_raw[:], in_=lay["rcp_t"][:, :])
        rcp_sb = sbuf.tile([P, ng], F32, name=f"rcp_sb{i}")
        nc.vector.tensor_copy(rcp_sb[:], rcp_raw[:])
        ones_sb = sbuf.tile([1, P], BF16, name=f"ones{i}")
        nc.vector.memset(ones_sb[:], 1.0)

        st = []
        for c in range(nt):
            t = sbuf.tile([P, SPr], BF16, name=f"st{i}_{c}")
            nc.sync.dma_start(out=t[:], in_=seg_t[0:SPr, c * P:(c + 1) * P],
                              transpose=True)
            st.append(t)
        rt = sbuf.tile([P, nt * SPr], BF16, name=f"rt{i}")
        nc.gpsimd.dma_gather(
            rt[:].rearrange("p (e n) -> p e n", n=SPr),
            table_ap, ridx_sb[:, :],
            SPr, SPr, cin, elem_step=cin, transpose=True, single_packet=False,
        )

        ng = -(-seg // P)
        for g in range(ng):
            gsz = min(P, seg - g * P)
            hA = psum.tile([P, cout], F32, tag=f"hA_p{i}", bufs=4,
                           name=f"hA_p{i}_{g}")
            hB = psum.tile([P, cout], F32, tag=f"hB_p{i}", bufs=4,
                           name=f"hB_p{i}_{g}")
            for c in range(nt):
                nc.tensor.matmul(hA[:], lhsT=st[c][:, g * P:(g + 1) * P],
                                 rhs=wl_tiles[c][:], start=(c == 0),
                                 stop=(c == nt - 1))
                nc.tensor.matmul(hB[:],
                                 lhsT=rt[:, c * SPr + g * P:c * SPr + (g + 1) * P],
                                 rhs=wr_tiles[c][:], start=(c == 0),
                                 stop=False)
            nc.tensor.matmul(hB[:], lhsT=ones_sb[:], rhs=b_row[:],
                             start=False, stop=True)
            if not log_softmax:
                o = sbuf.tile([P, cout], BF16, tag=f"o_p{i}", bufs=6,
                              name=f"o_p{i}_{g}")
                with nc.allow_low_precision(reason="bf16 mean scale + add"):
                    nc.vector.tensor_scalar_mul(o[:], hA[:],
                                                rcp_sb[:, g:g + 1])
                    nc.vector.tensor_tensor(out=o[:], in0=o[:], in1=hB[:],
                                            op=mybir.AluOpType.add)
                nc.sync.dma_start(out=out_slice[g * P:g * P + gsz, :],
                                  in_=o[:gsz, :])
            else:
                h_sb = sbuf.tile([P, cout], F32, tag="h_sb", bufs=2,
                                 name=f"h_sb{g}")
                nc.vector.tensor_scalar_mul(h_sb[:], hA[:], rcp_sb[:, g:g + 1])
                nc.vector.tensor_tensor(out=h_sb[:], in0=h_sb[:], in1=hB[:],
                                        op=mybir.AluOpType.add)
                negm = sbuf.tile([P, 1], F32, tag="negm", bufs=2,
                                 name=f"negm{g}")
                nc.vector.tensor_reduce(out=negm[:], in_=h_sb[:],
                                        axis=mybir.AxisListType.X,
                                        op=mybir.AluOpType.max, negate=True)
                esum = sbuf.tile([P, 1], F32, tag="esum", bufs=2,
                                 name=f"esum{g}")
                etile = sbuf.tile([P, cout], F32, tag="etile", bufs=2,
                                  name=f"etile{g}")
                nc.scalar.activation(out=etile[:], in_=h_sb[:],
                                     func=mybir.ActivationFunctionType.Exp,
                                     bias=negm[:], scale=1.0, accum_out=esum[:])
                lns = sbuf.tile([P, 1], F32, tag="lns", bufs=2,
                                name=f"lns{g}")
                nc.scalar.activation(out=lns[:], in_=esum[:],
                                     func=mybir.ActivationFunctionType.Ln)
                o = sbuf.tile([P, cout], F32, tag="o_ls", bufs=2,
                              name=f"o_ls{g}")
                nc.vector.tensor_scalar(
                    out=o[:], in0=h_sb[:], scalar1=negm[:], scalar2=lns[:],
                    op0=mybir.AluOpType.add, op1=mybir.AluOpType.subtract,
                )
                nc.sync.dma_start(out=out_ext[g * P:g * P + gsz, :],
                                  in_=o[:gsz, :])


# --------------------------------------------------------------------------- #
# program builder
# --------------------------------------------------------------------------- #

def _build_nc(meta):
    in_c, hid, out_c = meta["in_c"], meta["hid"], meta["out_c"]
    nc = bacc.Bacc("TRN2", target_bir_lowering=False, debug=False,
                   num_devices=NC, num_swdge_queues=2)

    TD0 = sum(meta["D0"])
    xs0 = nc.dram_tensor("xs0", [TD0 * P, in_c], BF16, kind="ExternalInput")
    xr0 = nc.dram_tensor("xr0", [SP0, in_c], BF16, kind="ExternalInput")
    rcp0 = nc.dram_tensor("rcp0", [P, G0], F32, kind="ExternalInput")
    msk0 = nc.dram_tensor("msk0", [SP0], BF16, kind="ExternalInput")
    iot = nc.dram_tensor("iot", [P, WIN * P], F32, kind="ExternalInput")
    din = {}
    for i, (nb, spr) in enumerate(((meta["NB1"], SP1), (meta["NB2"], SP2)),
                                  start=1):
        din[f"idx{i}"] = nc.dram_tensor(f"idx{i}", [P, nb * 8], I16,
                                        kind="ExternalInput")
        din[f"dr{i}"] = nc.dram_tensor(f"dr{i}", [P, nb], F32,
                                       kind="ExternalInput")
        din[f"ridx{i}"] = nc.dram_tensor(f"ridx{i}", [P, spr // 16], I16,
                                         kind="ExternalInput")
        din[f"rcp{i}"] = nc.dram_tensor(f"rcp{i}", [P, spr // P], F32,
                                        kind="ExternalInput")
    wl0 = nc.dram_tensor("wl0", [in_c, hid], BF16, kind="ExternalInput")
    wr0 = nc.dram_tensor("wr0", [in_c, hid], BF16, kind="ExternalInput")
    b0 = nc.dram_tensor("b0", [hid], BF16, kind="ExternalInput")
    wl1 = nc.dram_tensor("wl1", [hid, hid], BF16, kind="ExternalInput")
    wr1 = nc.dram_tensor("wr1", [hid, hid], BF16, kind="ExternalInput")
    b1 = nc.dram_tensor("b1", [hid], BF16, kind="ExternalInput")
    wl2 = nc.dram_tensor("wl2", [hid, out_c], BF16, kind="ExternalInput")
    wr2 = nc.dram_tensor("wr2", [hid, out_c], BF16, kind="ExternalInput")
    b2 = nc.dram_tensor("b2", [out_c], BF16, kind="ExternalInput")
    out = nc.dram_tensor("out", [S2, out_c], F32, kind="ExternalOutput")

    with tile.TileContext(nc) as tc:
        with (
            tc.tile_pool(name="const", bufs=1) as const,
            tc.tile_pool(name="dram", bufs=1, space="DRAM") as dram,
        ):
            ident_bf = const.tile([P, P], BF16)
            make_identity(nc, ident_bf[:])

            def load_w(t, rows, cols):
                tiles = []
                for i in range(-(-rows // P)):
                    ct = min(P, rows - i * P)
                    w_sb = const.tile([ct, cols], BF16, name=f"w_{t.name}_{i}")
                    nc.sync.dma_start(out=w_sb[:], in_=t[i * P:i * P + ct, :])
                    tiles.append(w_sb)
                return tiles

            wl0_t = load_w(wl0, in_c, hid)[0]
            wr0_t = load_w(wr0, in_c, hid)[0]
            wl1_t, wr1_t = load_w(wl1, hid, hid), load_w(wr1, hid, hid)
            wl2_t, wr2_t = load_w(wl2, hid, out_c), load_w(wr2, hid, out_c)
            b0_sb = const.tile([1, hid], BF16)
            nc.sync.dma_start(out=b0_sb[:], in_=b0[None, :])
            b1_sb = const.tile([1, hid], BF16)
            nc.sync.dma_start(out=b1_sb[:], in_=b1[None, :])
            b2_sb = const.tile([1, out_c], BF16)
            nc.sync.dma_start(out=b2_sb[:], in_=b2[None, :])

            pl1, pl2 = meta["pl1"], meta["pl2"]
            h1_slice = dram.tile([SP0, hid], BF16)
            partials1 = [dram.tile([pl1["crows"], hid], BF16,
                                   name=f"partial1_{c}")
                         for c in range(pl1["rs_chunks"])]
            seg1 = dram.tile([SP1, hid], BF16)
            h2_slice = dram.tile([SP1, hid], BF16)
            partials2 = [dram.tile([pl2["crows"], hid], BF16,
                                   name=f"partial2_{c}")
                         for c in range(pl2["rs_chunks"])]
            seg2 = dram.tile([SP2, hid], BF16)

            # zero the padding rows of h2_slice (layer-2 gather zero rows)
            zpad = const.tile([SP1 - S1, hid], BF16)
            nc.vector.memset(zpad[:], 0.0)
            nc.sync.dma_start(out=h2_slice[S1:SP1, :], in_=zpad[:])

            _emit_l0(nc, tc, meta, xs0, xr0, rcp0, msk0, h1_slice,
                     wl0_t, wr0_t, b0_sb, ident_bf)

            lay1 = dict(i=1, pl=pl1, NB=pl1["NB"],
                        idx_t=din["idx1"], dr_t=din["dr1"], iot_t=iot,
                        ridx_t=din["ridx1"], rcp_t=din["rcp1"])
            def rs1_emit(c):
                nc.gpsimd.collective_compute(
                    "ReduceScatter", mybir.AluOpType.add,
                    replica_groups=[list(range(NC))],
                    ins=[partials1[c][:]],
                    outs=[seg1[c * pl1["piece"]:(c + 1) * pl1["piece"], :]],
                )
            _emit_agg(nc, tc, lay1, h1_slice[:, :], partials1, hid,
                      rs_emit=rs1_emit)
            _emit_post(nc, tc, lay1, seg1, h1_slice[:, :], h2_slice,
                       wl1_t, wr1_t, b1_sb, hid, hid, S1, SP0)

            lay2 = dict(i=2, pl=pl2, NB=pl2["NB"],
                        idx_t=din["idx2"], dr_t=din["dr2"], iot_t=iot,
                        ridx_t=din["ridx2"], rcp_t=din["rcp2"])
            def rs2_emit(c):
                nc.gpsimd.collective_compute(
                    "ReduceScatter", mybir.AluOpType.add,
                    replica_groups=[list(range(NC))],
                    ins=[partials2[c][:]],
                    outs=[seg2[c * pl2["piece"]:(c + 1) * pl2["piece"], :]],
                )
            _emit_agg(nc, tc, lay2, h2_slice[:, :], partials2, hid,
                      rs_emit=rs2_emit)
            _emit_post(nc, tc, lay2, seg2, h2_slice[:, :], None,
                       wl2_t, wr2_t, b2_sb, hid, out_c, S2, SP1,
                       log_softmax=True, out_ext=out)
    nc.finalize()
    return nc


# --------------------------------------------------------------------------- #
# entry point
# --------------------------------------------------------------------------- #

def _prepare(x, src0, dst0, src1, dst1, src2, dst2, n1, n2, n3,
             Wl0, Wr0, b0, Wl1, Wr1, b1, Wl2, Wr2, b2):
    import ml_dtypes
    BF = ml_dtypes.bfloat16
    x16 = np.asarray(x, np.float32).astype(BF)
    src0, dst0 = np.asarray(src0, np.int64), np.asarray(dst0, np.int64)
    src1, dst1 = np.asarray(src1, np.int64), np.asarray(dst1, np.int64)
    src2, dst2 = np.asarray(src2, np.int64), np.asarray(dst2, np.int64)
    assert (int(n1), int(n2), int(n3)) == (N1, N2, N3)

    p0 = _plan_l0(src0, dst0, N1)
    row2 = _owner(np.arange(N2V)) * SP1 + _lpos(np.arange(N2V))
    pl1 = _plan_src_layer(src1, dst1, N2V, p0["row1"], SP0, S0, S1, G1T,
                          rs_chunks=7)
    pl2 = _plan_src_layer(src2, dst2, N3, row2, SP1, S1, S2, G2T,
                          rs_chunks=1)

    meta = dict(
        in_c=x16.shape[1], hid=Wl0.shape[1], out_c=Wl2.shape[1],
        D0=p0["D"], pl1=pl1, pl2=pl2, NB1=pl1["NB"], NB2=pl2["NB"],
    )
    iot = np.tile(np.arange(WIN * P, dtype=np.float32)[None, :], (P, 1))
    in_maps = []
    for k in range(NC):
        xs, xr, rcp, msk = _l0_tables(p0, k, x16)
        m = dict(
            xs0=xs, xr0=xr, rcp0=np.ascontiguousarray(rcp),
            msk0=msk.astype(BF), iot=iot,
            wl0=np.asarray(Wl0, np.float32).astype(BF),
            wr0=np.asarray(Wr0, np.float32).astype(BF),
            b0=np.asarray(b0, np.float32).astype(BF),
            wl1=np.asarray(Wl1, np.float32).astype(BF),
            wr1=np.asarray(Wr1, np.float32).astype(BF),
            b1=np.asarray(b1, np.float32).astype(BF),
            wl2=np.asarray(Wl2, np.float32).astype(BF),
            wr2=np.asarray(Wr2, np.float32).astype(BF),
            b2=np.asarray(b2, np.float32).astype(BF),
        )
        for i, pl in ((1, pl1), (2, pl2)):
            m[f"idx{i}"] = _pack_idx(pl["idx"][k])
            m[f"dr{i}"] = np.ascontiguousarray(
                pl["dr"][k].reshape(-1, P).T.astype(np.float32))
            m[f"ridx{i}"] = _pack_idx(pl["ridx"][k])
            m[f"rcp{i}"] = np.ascontiguousarray(
                pl["rcp"][k].reshape(-1, P).T)
        in_maps.append(m)
    return (p0, pl1, pl2), meta, in_maps


def _assemble(outs):
    t = np.arange(N3)
    full = np.stack(outs)  # [NC, S2, out_c]
    return np.ascontiguousarray(full[_owner(t), _lpos(t)])


def kernel(**inputs) -> np.ndarray:
    from concourse.bass_utils import run_bass_kernel_spmd

    _, meta, in_maps = _prepare(**inputs)
    nc = _build_nc(meta)
    res = run_bass_kernel_spmd(nc, in_maps, core_ids=list(range(NC)))
    return _assemble([res.results[k]["out"] for k in range(NC)])


# revision 8
# speedup vs baseline: 1.0447x; 1.0086x over previous
"""BinSAGE v2 (3-layer bipartite GraphSAGE, mean aggregation) on 8 TRN2 cores.

Sharding:
- Node spaces are interleaved across cores in blocks of 4 ids
  (owner(id) = (id//4) % 8), which makes every layer's root features local.
- Layer 0 is dst-sharded: per core, targets are degree-sorted and packed
  into ELL groups of 128; the host pre-gathers the neighbor feature stream
  (one dense DMA per group), the device does a contiguous DVE tree-add
  segment-sum, then the SAGE transform on the PE.
- Layers 1/2 are src-sharded: each core holds the edges whose SOURCE row
  lives in its local feature table, gathers messages with a few big
  dma_gather instructions (994ns fixed cost amortized over thousands of
  rows), one-hot matmuls accumulate partial target sums in canonical
  (owner-major) order, and a bf16 ReduceScatter sums partials across cores.
  Post-collective, each core fetches its (local) root rows with a
  transposed dma_gather, loads the scattered segment with a DMA transpose,
  and runs the transform + (for the last layer) log_softmax.
"""

import numpy as np

import concourse.bass as bass
import concourse.bacc as bacc
import concourse.mybir as mybir
import concourse.tile as tile
from concourse.masks import make_identity

NC = 8
P = 128
IL = 4  # interleave block (ids i: owner = (i//IL) % NC)
F32 = mybir.dt.float32
BF16 = mybir.dt.bfloat16
I16 = mybir.dt.int16

CHUNK_BLOCKS = 28  # gather chunk size (blocks of 128 rows x 512B)
WIN = 4  # target groups per window (512 targets)

IN_C, HID, OUT_C = 100, 256, 47
N0, N1, N2, N3 = 500000, 100000, 25000, 4096
N2V = 25088  # virtual layer-1 target space (ids >= N2 have no edges)
S0, G0, SP0 = 12500, 98, 12544     # layer-0 per-core targets / groups / rows
S1, G1T, SP1 = 3136, 196, 3200     # layer-1 per-core targets / global groups
S2, G2T, SP2 = 512, 32, 512        # layer-2


def _owner(ids):
    return (ids // IL) % NC


def _lpos(ids):
    return (ids // (IL * NC)) * IL + ids % IL


def _tgt_of(k, j):
    """Inverse of (owner, lpos) for target id."""
    return (j // IL) * (IL * NC) + k * IL + j % IL


# --------------------------------------------------------------------------- #
# host planning
# --------------------------------------------------------------------------- #

def _plan_l0(src0, dst0, n1):
    s, G, SP = S0, G0, SP0
    deg = np.bincount(dst0, minlength=n1)
    eo = _owner(dst0)
    ids = np.arange(n1)
    own_t = _owner(ids)
    per_core = []
    Dmax = np.zeros(G, np.int64)
    row1 = np.empty(n1, np.int64)
    slot_of = np.empty(n1, np.int64)
    for k in range(NC):
        tids = ids[own_t == k]                     # ascending, len s
        order = np.argsort(-deg[tids], kind="stable")
        slot_t = tids[order]                       # slot i -> target id
        row1[slot_t] = k * SP + np.arange(s)
        slot_of[slot_t] = np.arange(s)
        em = eo == k
        es, ed = src0[em], dst0[em]
        eslot = slot_of[ed]
        eord = np.argsort(eslot, kind="stable")
        csr_src = es[eord]
        starts = np.zeros(s + 1, np.int64)
        np.cumsum(np.bincount(eslot, minlength=s), out=starts[1:])
        sdeg = deg[slot_t]                         # descending
        gmax = np.array([sdeg[g * P] if g * P < s else 0 for g in range(G)])
        Dmax = np.maximum(Dmax, gmax)
        per_core.append((slot_t, csr_src, starts, sdeg))
    return dict(D=[int(d) for d in Dmax], per_core=per_core, row1=row1)


def _l0_tables(p0, k, x16):
    """Stage core k's ELL stream + roots + rcp + msk."""
    D, (slot_t, csr_src, starts, sdeg) = p0["D"], p0["per_core"][k]
    s, G, SP = S0, G0, SP0
    in_c = x16.shape[1]
    tot = sum(D)
    xs = np.zeros((tot * P, in_c), x16.dtype)
    off = 0
    for g in range(G):
        Dg = D[g]
        if Dg == 0:
            continue
        n = min(P, s - g * P)
        j = np.arange(Dg)[None, :]
        st = starts[g * P:g * P + n][:, None]
        dg = sdeg[g * P:g * P + n][:, None]
        valid = j < dg
        pos = np.where(valid, st + j, 0)
        seg = np.zeros((P, Dg, in_c), x16.dtype)
        rows = csr_src[pos]
        seg[:n][valid] = x16[rows[valid]]
        # [P, in_c, Dg] so the device reduce is innermost-contiguous
        xs[off * P:(off + Dg) * P] = np.ascontiguousarray(
            seg.transpose(0, 2, 1)).reshape(P * Dg, in_c)
        off += Dg
    xr = np.zeros((SP, in_c), x16.dtype)
    xr[:s] = x16[slot_t]
    rcp = np.ones((P, G), np.float32)
    sd = np.concatenate([sdeg, np.zeros(SP - s, np.int64)])
    rcp[:, :] = (1.0 / np.maximum(sd.reshape(G, P), 1)).T
    msk = np.zeros(SP, np.float32)
    msk[:s] = 1.0
    return xs, xr, rcp, msk


def _plan_src_layer(src, dst, n_tgt, row_src, sp_src, zero_row, seg, Gt,
                    rs_chunks=1):
    """Src-sharded layer, window-packed blocks (WIN groups per window),
    chunk-major canonical prow order so the ReduceScatter can be split into
    rs_chunks overlapping collectives.  Blocks are packed densely within a
    window; the common (union) schedule records which groups each block
    touches."""
    tids = np.arange(n_tgt)
    own_t, lp_t = _owner(tids), _lpos(tids)
    piece = seg // rs_chunks
    crows = piece * NC
    prow_t = (lp_t // piece) * crows + own_t * piece + (lp_t % piece)
    eo = _owner(src)
    cnt_global = np.bincount(dst, minlength=n_tgt)
    NW = Gt // WIN
    counts = np.zeros((NC, NW), np.int64)
    core_edges = []
    for k in range(NC):
        m = eo == k
        es, ep = src[m], prow_t[dst[m]]
        o = np.argsort(ep, kind="stable")
        es, ep = es[o], ep[o]
        counts[k] = np.bincount(ep // (WIN * P), minlength=NW)
        core_edges.append((es, ep))
    BW = np.maximum(-(-counts // P), 1).max(axis=0)   # blocks per window
    NB = int(BW.sum())
    woff = np.zeros(NW + 1, np.int64)
    np.cumsum(BW, out=woff[1:])
    # union touch map: touch[b] = set of in-window groups any core hits
    touch = [set() for _ in range(NB)]
    for k in range(NC):
        es, ep = core_edges[k]
        west = np.zeros(NW + 1, np.int64)
        np.cumsum(counts[k], out=west[1:])
        for w in range(NW):
            e0, e1 = west[w], west[w + 1]
            if e1 == e0:
                continue
            gw = (ep[e0:e1] - w * WIN * P) // P
            bl = np.arange(e1 - e0) // P
            for b in range(int(bl[-1]) + 1):
                for g in np.unique(gw[bl == b]):
                    touch[woff[w] + b].add(int(g))
    gmin = np.zeros(NB, np.int64)
    span = np.ones(NB, np.int64)
    for b in range(NB):
        if touch[b]:
            gmin[b] = min(touch[b])
            span[b] = max(touch[b]) - gmin[b] + 1
    # per-window, per-group ordered block lists (window-local block ids)
    sched = []
    for w in range(NW):
        sw = []
        for g in range(WIN):
            sw.append([b for b in range(int(BW[w]))
                       if g in touch[woff[w] + b]])
        sched.append(sw)
    # chunks of whole windows, <= CHUNK_BLOCKS blocks each
    chunks, cur, cb = [], [], 0
    for w in range(NW):
        if cur and cb + BW[w] > CHUNK_BLOCKS:
            chunks.append(cur)
            cur, cb = [], 0
        cur.append(w)
        cb += BW[w]
    if cur:
        chunks.append(cur)
    woh = [int(max(span[woff[w]:woff[w + 1]].max() for w in ws))
           for ws in chunks]
    # per-core tables
    idx_list, dr_list, rcp_list, ridx_list, ep_list = [], [], [], [], []
    for k in range(NC):
        es, ep = core_edges[k]
        idx = np.full(NB * P, zero_row, np.int64)
        drl = np.full(NB * P, 2000.0, np.float32)
        epl = np.full(NB * P, -1, np.int64)
        west = np.zeros(NW + 1, np.int64)
        np.cumsum(counts[k], out=west[1:])
        for w in range(NW):
            e0, e1 = west[w], west[w + 1]
            if e1 == e0:
                continue
            n = e1 - e0
            pos = woff[w] * P + np.arange(n)
            idx[pos] = row_src[es[e0:e1]] - k * sp_src
            dw = ep[e0:e1] - w * WIN * P
            bl = np.arange(n) // P
            drl[pos] = dw - gmin[woff[w] + bl] * P   # shifted in-window drel
            epl[pos] = ep[e0:e1]
        t_loc = _tgt_of(k, np.arange(seg))
        ridx = np.full(-(-seg // P) * P, zero_row, np.int64)
        ridx[:seg] = row_src[t_loc] - k * sp_src
        rcp = np.ones(-(-seg // P) * P, np.float32)
        rcp[:seg] = 1.0 / np.maximum(cnt_global[t_loc], 1)
        idx_list.append(idx)
        dr_list.append(drl)
        rcp_list.append(rcp)
        ridx_list.append(ridx)
        ep_list.append(epl)
    return dict(BW=[int(b) for b in BW], NB=NB, woff=woff, chunks=chunks,
                woh=woh, sched=sched, gmin=gmin, rs_chunks=rs_chunks,
                piece=piece, crows=crows, NW=NW,
                idx=idx_list, dr=dr_list, rcp=rcp_list, ridx=ridx_list,
                eprow=ep_list)


def _pack_idx(idx):
    """idx list (len % 16 == 0) -> [128, n/16] int16 (i at [i%16, i//16])."""
    t = np.asarray(idx, np.int16).reshape(-1, 16).T
    return np.ascontiguousarray(np.tile(t, (8, 1)))


# --------------------------------------------------------------------------- #
# device emitters
# --------------------------------------------------------------------------- #

def _emit_l0(nc, tc, meta, xs0, xr0, rcp0, msk0, h1_slice,
             wl_t, wr_t, b_sb, ident_bf):
    in_c, hid = meta["in_c"], meta["hid"]
    D = meta["D0"]
    with (
        tc.tile_pool(name="l0_sbuf", bufs=1) as sbuf,
        tc.tile_pool(name="l0_psum", bufs=1, space="PSUM") as psum,
    ):
        rcp_raw = sbuf.tile([P, G0], F32, name="rcp_raw0")
        nc.sync.dma_start(out=rcp_raw[:], in_=rcp0[:, :])
        rcp_sb = sbuf.tile([P, G0], F32, name="rcp_sb0")
        nc.vector.tensor_copy(rcp_sb[:], rcp_raw[:])
        msk_sb = sbuf.tile([1, SP0], BF16, name="msk_sb0")
        nc.sync.dma_start(out=msk_sb[:], in_=msk0[None, :])

        off = 0
        for g in range(G0):
            Dg = D[g]
            mean = sbuf.tile([P, in_c], BF16, tag="mean0", bufs=6,
                             name=f"mean0_{g}")
            if Dg > 0:
                # xs0 is staged [P, in_c, Dg] per group: the segment-sum is
                # one contiguous innermost-axis reduce.
                msg = sbuf.tile([P, in_c * Dg], BF16, tag="msg0", bufs=5,
                                name=f"msg0_{g}")
                nc.sync.dma_start(
                    out=msg[:],
                    in_=xs0[off * P:(off + Dg) * P, :]
                        .rearrange("(p j) c -> p (j c)", p=P),
                )
                ssum = sbuf.tile([P, in_c], BF16, tag="ssum0", bufs=6,
                                 name=f"ssum0_{g}")
                with nc.allow_low_precision(reason="bf16 neighbor sum"):
                    nc.vector.tensor_reduce(
                        out=ssum[:],
                        in_=msg[:].rearrange("p (c j) -> p c j", j=Dg),
                        axis=mybir.AxisListType.X,
                        op=mybir.AluOpType.add,
                    )
                nc.scalar.activation(out=mean[:], in_=ssum[:],
                                     func=mybir.ActivationFunctionType.Copy,
                                     scale=rcp_sb[:, g:g + 1])
            else:
                nc.vector.memset(mean[:], 0.0)
            root = sbuf.tile([P, in_c], BF16, tag="root0", bufs=6,
                             name=f"root0_{g}")
            nc.sync.dma_start(out=root[:], in_=xr0[g * P:(g + 1) * P, :])

            h_ps = psum.tile([P, hid], F32, tag="hps0", bufs=3,
                             name=f"hps0_{g}")
            first = True
            for tin, w in ((mean, wl_t), (root, wr_t)):
                tp = psum.tile([in_c, P], BF16, tag="tp0", bufs=4,
                               name=f"tp0_{g}_{id(w)}")
                nc.tensor.transpose(out=tp[:], in_=tin[:], identity=ident_bf[:])
                tps = sbuf.tile([in_c, P], BF16, tag="tps0", bufs=6,
                                name=f"tps0_{g}_{id(w)}")
                nc.scalar.copy(tps[:], tp[:])
                nc.tensor.matmul(h_ps[:], lhsT=tps[:], rhs=w[:],
                                 start=first, stop=False)
                first = False
            nc.tensor.matmul(h_ps[:], lhsT=msk_sb[:, g * P:(g + 1) * P],
                             rhs=b_sb[:], start=False, stop=True)
            o = sbuf.tile([P, hid], BF16, tag="o0", bufs=6, name=f"o0_{g}")
            nc.scalar.copy(o[:], h_ps[:])
            nc.scalar.dma_start(out=h1_slice[g * P:(g + 1) * P, :], in_=o[:])
            off += Dg


def _emit_agg(nc, tc, lay, table_ap, partials, cin, rs_emit=None):
    """Src-sharded partial aggregation: chunked dma_gather + one-hot matmuls
    per (window, group); one batched partial write per window."""
    pl = lay["pl"]
    BW, woff, chunks, woh = pl["BW"], pl["woff"], pl["chunks"], pl["woh"]
    sched, gmin, NB = pl["sched"], pl["gmin"], pl["NB"]
    i = lay["i"]
    with (
        tc.tile_pool(name=f"agg{i}_sbuf", bufs=1) as sbuf,
        tc.tile_pool(name=f"agg{i}_psum", bufs=1, space="PSUM") as psum,
    ):
        idx_sb = sbuf.tile([P, NB * 8], I16, name=f"idx_sb{i}")
        nc.sync.dma_start(out=idx_sb[:], in_=lay["idx_t"][:, :])
        dr_raw = sbuf.tile([P, NB], F32, name=f"dr_raw{i}")
        nc.sync.dma_start(out=dr_raw[:], in_=lay["dr_t"][:, :])
        dr_sb = sbuf.tile([P, NB], F32, name=f"dr_sb{i}")
        nc.vector.tensor_copy(dr_sb[:], dr_raw[:])
        iot_sb = sbuf.tile([P, WIN * P], F32, name=f"iot_sb{i}")
        nc.sync.dma_start(out=iot_sb[:], in_=lay["iot_t"][:, :])

        nwpc = pl["crows"] // (WIN * P)   # windows per RS chunk
        for ci, ws in enumerate(chunks):
            b0 = int(woff[ws[0]])
            nb = sum(BW[w] for w in ws)
            wo = woh[ci] * P
            msg = sbuf.tile([P, nb * cin], BF16, tag=f"msg{i}", bufs=6,
                            name=f"msg{i}_{ci}")
            nc.gpsimd.dma_gather(
                msg[:].rearrange("p (b c) -> p b c", c=cin),
                table_ap,
                idx_sb[:, b0 * 8:(b0 + nb) * 8],
                nb * P, nb * P, cin, elem_step=cin, single_packet=False,
                queue_num=ci % 2,
            )
            oh = sbuf.tile([P, nb * wo], BF16, tag=f"oh{i}", bufs=4,
                           name=f"oh{i}_{ci}")
            nc.vector.tensor_tensor(
                out=oh[:].rearrange("p (r c) -> p r c", c=wo),
                in0=dr_sb[:, b0:b0 + nb]
                    .rearrange("p (r u) -> p r u", u=1)
                    .to_broadcast([P, nb, wo]),
                in1=iot_sb[:, :wo].rearrange("p (u c) -> p u c", u=1)
                    .to_broadcast([P, nb, wo]),
                op=mybir.AluOpType.is_equal,
            )
            bb = 0
            for w in ws:
                po = sbuf.tile([P, WIN * cin], BF16, tag=f"po{i}", bufs=6,
                               name=f"po{i}_{w}")
                for g in range(WIN):
                    blocks = sched[w][g]
                    if not blocks:
                        nc.vector.memset(po[:, g * cin:(g + 1) * cin], 0.0)
                        continue
                    h_ps = psum.tile([P, cin], F32, tag=f"hps{i}", bufs=8,
                                     name=f"hps{i}_{w}_{g}")
                    for j, b in enumerate(blocks):
                        gb = woff[w] + b        # global block id
                        col = (g - int(gmin[gb])) * P
                        nc.tensor.matmul(
                            h_ps[:],
                            lhsT=oh[:, (bb + b) * wo + col:
                                    (bb + b) * wo + col + P],
                            rhs=msg[:, (bb + b) * cin:(bb + b + 1) * cin],
                            start=(j == 0), stop=(j == len(blocks) - 1))
                    with nc.allow_low_precision(reason="bf16 partials"):
                        nc.scalar.copy(po[:, g * cin:(g + 1) * cin], h_ps[:])
                part = partials[w // nwpc]
                r0 = (w % nwpc) * WIN * P
                nc.scalar.dma_start(
                    out=part[r0:r0 + WIN * P, :]
                        .rearrange("(b p) c -> p b c", p=P),
                    in_=po[:].rearrange("p (b c) -> p b c", c=cin),
                )
                if rs_emit is not None and (w + 1) % nwpc == 0:
                    rs_emit(w // nwpc)
                bb += BW[w]


def _emit_post(nc, tc, lay, seg_t, table_ap, out_slice, wl_tiles, wr_tiles,
               b_row, cin, cout, seg, sp, log_softmax=False, out_ext=None):
    """Post-RS: transposed loads + root gather + transform (+ log_softmax)."""
    i = lay["i"]
    nt = cin // P  # 2
    SPr = -(-seg // P) * P  # root-gather rows (3200 / 512)
    with (
        tc.tile_pool(name=f"post{i}_sbuf", bufs=1) as sbuf,
        tc.tile_pool(name=f"post{i}_psum", bufs=1, space="PSUM") as psum,
    ):
        ng = SPr // P
        ridx_sb = sbuf.tile([P, SPr // 16], I16, name=f"ridx_sb{i}")
        nc.sync.dma_start(out=ridx_sb[:], in_=lay["ridx_t"][:, :])
        rcp_raw = sbuf.tile([P, ng], F32, name=f"rcp_raw{i}")
        nc.sync.dma_start(out=rcp_raw[:], in_=lay["rcp_t"][:, :])
        rcp_sb = sbuf.tile([P, ng], F32, name=f"rcp_sb{i}")
        nc.vector.tensor_copy(rcp_sb[:], rcp_raw[:])
        ones_sb = sbuf.tile([1, P], BF16, name=f"ones{i}")
        nc.vector.memset(ones_sb[:], 1.0)

        st = []
        for c in range(nt):
            t = sbuf.tile([P, SPr], BF16, name=f"st{i}_{c}")
            nc.sync.dma_start(out=t[:], in_=seg_t[0:SPr, c * P:(c + 1) * P],
                              transpose=True)
            st.append(t)
        rt = sbuf.tile([P, nt * SPr], BF16, name=f"rt{i}")
        nc.gpsimd.dma_gather(
            rt[:].rearrange("p (e n) -> p e n", n=SPr),
            table_ap, ridx_sb[:, :],
            SPr, SPr, cin, elem_step=cin, transpose=True, single_packet=False,
        )

        ng = -(-seg // P)
        for g in range(ng):
            gsz = min(P, seg - g * P)
            hA = psum.tile([P, cout], F32, tag=f"hA_p{i}", bufs=4,
                           name=f"hA_p{i}_{g}")
            hB = psum.tile([P, cout], F32, tag=f"hB_p{i}", bufs=4,
                           name=f"hB_p{i}_{g}")
            for c in range(nt):
                nc.tensor.matmul(hA[:], lhsT=st[c][:, g * P:(g + 1) * P],
                                 rhs=wl_tiles[c][:], start=(c == 0),
                                 stop=(c == nt - 1))
                nc.tensor.matmul(hB[:],
                                 lhsT=rt[:, c * SPr + g * P:c * SPr + (g + 1) * P],
                                 rhs=wr_tiles[c][:], start=(c == 0),
                                 stop=False)
            nc.tensor.matmul(hB[:], lhsT=ones_sb[:], rhs=b_row[:],
                             start=False, stop=True)
            if not log_softmax:
                o = sbuf.tile([P, cout], BF16, tag=f"o_p{i}", bufs=6,
                              name=f"o_p{i}_{g}")
                with nc.allow_low_precision(reason="bf16 mean scale + add"):
                    nc.vector.tensor_scalar_mul(o[:], hA[:],
                                                rcp_sb[:, g:g + 1])
                    nc.vector.tensor_tensor(out=o[:], in0=o[:], in1=hB[:],
                                            op=mybir.AluOpType.add)
                nc.sync.dma_start(out=out_slice[g * P:g * P + gsz, :],
                                  in_=o[:gsz, :])
            else:
                h_sb = sbuf.tile([P, cout], F32, tag="h_sb", bufs=2,
                                 name=f"h_sb{g}")
                nc.vector.tensor_scalar_mul(h_sb[:], hA[:], rcp_sb[:, g:g + 1])
                nc.vector.tensor_tensor(out=h_sb[:], in0=h_sb[:], in1=hB[:],
                                        op=mybir.AluOpType.add)
                negm = sbuf.tile([P, 1], F32, tag="negm", bufs=2,
                                 name=f"negm{g}")
                nc.vector.tensor_reduce(out=negm[:], in_=h_sb[:],
                                        axis=mybir.AxisListType.X,
                                        op=mybir.AluOpType.max, negate=True)
                esum = sbuf.tile([P, 1], F32, tag="esum", bufs=2,
                                 name=f"esum{g}")
                etile = sbuf.tile([P, cout], F32, tag="etile", bufs=2,
                                  name=f"etile{g}")
                nc.scalar.activation(out=etile[:], in_=h_sb[:],
                                     func=mybir.ActivationFunctionType.Exp,
                                     bias=negm[:], scale=1.0, accum_out=esum[:])
                lns = sbuf.tile([P, 1], F32, tag="lns", bufs=2,
                                name=f"lns{g}")
                nc.scalar.activation(out=lns[:], in_=esum[:],
                                     func=mybir.ActivationFunctionType.Ln)
                o = sbuf.tile([P, cout], F32, tag="o_ls", bufs=2,
                              name=f"o_ls{g}")
                nc.vector.tensor_scalar(
                    out=o[:], in0=h_sb[:], scalar1=negm[:], scalar2=lns[:],
                    op0=mybir.AluOpType.add, op1=mybir.AluOpType.subtract,
                )
                nc.sync.dma_start(out=out_ext[g * P:g * P + gsz, :],
                                  in_=o[:gsz, :])


# --------------------------------------------------------------------------- #
# program builder
# --------------------------------------------------------------------------- #

def _build_nc(meta):
    in_c, hid, out_c = meta["in_c"], meta["hid"], meta["out_c"]
    nc = bacc.Bacc("TRN2", target_bir_lowering=False, debug=False,
                   num_devices=NC, num_swdge_queues=2)

    TD0 = sum(meta["D0"])
    xs0 = nc.dram_tensor("xs0", [TD0 * P, in_c], BF16, kind="ExternalInput")
    xr0 = nc.dram_tensor("xr0", [SP0, in_c], BF16, kind="ExternalInput")
    rcp0 = nc.dram_tensor("rcp0", [P, G0], F32, kind="ExternalInput")
    msk0 = nc.dram_tensor("msk0", [SP0], BF16, kind="ExternalInput")
    iot = nc.dram_tensor("iot", [P, WIN * P], F32, kind="ExternalInput")
    din = {}
    for i, (nb, spr) in enumerate(((meta["NB1"], SP1), (meta["NB2"], SP2)),
                                  start=1):
        din[f"idx{i}"] = nc.dram_tensor(f"idx{i}", [P, nb * 8], I16,
                                        kind="ExternalInput")
        din[f"dr{i}"] = nc.dram_tensor(f"dr{i}", [P, nb], F32,
                                       kind="ExternalInput")
        din[f"ridx{i}"] = nc.dram_tensor(f"ridx{i}", [P, spr // 16], I16,
                                         kind="ExternalInput")
        din[f"rcp{i}"] = nc.dram_tensor(f"rcp{i}", [P, spr // P], F32,
                                        kind="ExternalInput")
    wl0 = nc.dram_tensor("wl0", [in_c, hid], BF16, kind="ExternalInput")
    wr0 = nc.dram_tensor("wr0", [in_c, hid], BF16, kind="ExternalInput")
    b0 = nc.dram_tensor("b0", [hid], BF16, kind="ExternalInput")
    wl1 = nc.dram_tensor("wl1", [hid, hid], BF16, kind="ExternalInput")
    wr1 = nc.dram_tensor("wr1", [hid, hid], BF16, kind="ExternalInput")
    b1 = nc.dram_tensor("b1", [hid], BF16, kind="ExternalInput")
    wl2 = nc.dram_tensor("wl2", [hid, out_c], BF16, kind="ExternalInput")
    wr2 = nc.dram_tensor("wr2", [hid, out_c], BF16, kind="ExternalInput")
    b2 = nc.dram_tensor("b2", [out_c], BF16, kind="ExternalInput")
    out = nc.dram_tensor("out", [S2, out_c], F32, kind="ExternalOutput")

    with tile.TileContext(nc) as tc:
        with (
            tc.tile_pool(name="const", bufs=1) as const,
            tc.tile_pool(name="dram", bufs=1, space="DRAM") as dram,
        ):
            ident_bf = const.tile([P, P], BF16)
            make_identity(nc, ident_bf[:])

            def load_w(t, rows, cols):
                tiles = []
                for i in range(-(-rows // P)):
                    ct = min(P, rows - i * P)
                    w_sb = const.tile([ct, cols], BF16, name=f"w_{t.name}_{i}")
                    nc.sync.dma_start(out=w_sb[:], in_=t[i * P:i * P + ct, :])
                    tiles.append(w_sb)
                return tiles

            wl0_t = load_w(wl0, in_c, hid)[0]
            wr0_t = load_w(wr0, in_c, hid)[0]
            wl1_t, wr1_t = load_w(wl1, hid, hid), load_w(wr1, hid, hid)
            wl2_t, wr2_t = load_w(wl2, hid, out_c), load_w(wr2, hid, out_c)
            b0_sb = const.tile([1, hid], BF16)
            nc.sync.dma_start(out=b0_sb[:], in_=b0[None, :])
            b1_sb = const.tile([1, hid], BF16)
            nc.sync.dma_start(out=b1_sb[:], in_=b1[None, :])
            b2_sb = const.tile([1, out_c], BF16)
            nc.sync.dma_start(out=b2_sb[:], in_=b2[None, :])

            pl1, pl2 = meta["pl1"], meta["pl2"]
            h1_slice = dram.tile([SP0, hid], BF16)
            partials1 = [dram.tile([pl1["crows"], hid], BF16,
                                   name=f"partial1_{c}")
                         for c in range(pl1["rs_chunks"])]
            seg1 = dram.tile([SP1, hid], BF16)
            h2_slice = dram.tile([SP1, hid], BF16)
            partials2 = [dram.tile([pl2["crows"], hid], BF16,
                                   name=f"partial2_{c}")
                         for c in range(pl2["rs_chunks"])]
            seg2 = dram.tile([SP2, hid], BF16)

            # zero the padding rows of h2_slice (layer-2 gather zero rows)
            zpad = const.tile([SP1 - S1, hid], BF16)
            nc.vector.memset(zpad[:], 0.0)
            nc.sync.dma_start(out=h2_slice[S1:SP1, :], in_=zpad[:])

            _emit_l0(nc, tc, meta, xs0, xr0, rcp0, msk0, h1_slice,
                     wl0_t, wr0_t, b0_sb, ident_bf)

            lay1 = dict(i=1, pl=pl1, NB=pl1["NB"],
                        idx_t=din["idx1"], dr_t=din["dr1"], iot_t=iot,
                        ridx_t=din["ridx1"], rcp_t=din["rcp1"])
            def rs1_emit(c):
                nc.gpsimd.collective_compute(
                    "ReduceScatter", mybir.AluOpType.add,
                    replica_groups=[list(range(NC))],
                    ins=[partials1[c][:]],
                    outs=[seg1[c * pl1["piece"]:(c + 1) * pl1["piece"], :]],
                )
            _emit_agg(nc, tc, lay1, h1_slice[:, :], partials1, hid,
                      rs_emit=rs1_emit)
            _emit_post(nc, tc, lay1, seg1, h1_slice[:, :], h2_slice,
                       wl1_t, wr1_t, b1_sb, hid, hid, S1, SP0)

            lay2 = dict(i=2, pl=pl2, NB=pl2["NB"],
                        idx_t=din["idx2"], dr_t=din["dr2"], iot_t=iot,
                        ridx_t=din["ridx2"], rcp_t=din["rcp2"])
            def rs2_emit(c):
                nc.gpsimd.collective_compute(
                    "ReduceScatter", mybir.AluOpType.add,
                    replica_groups=[list(range(NC))],
                    ins=[partials2[c][:]],
                    outs=[seg2[c * pl2["piece"]:(c + 1) * pl2["piece"], :]],
                )
            _emit_agg(nc, tc, lay2, h2_slice[:, :], partials2, hid,
                      rs_emit=rs2_emit)
            _emit_post(nc, tc, lay2, seg2, h2_slice[:, :], None,
                       wl2_t, wr2_t, b2_sb, hid, out_c, S2, SP1,
                       log_softmax=True, out_ext=out)
    nc.finalize()
    return nc


# --------------------------------------------------------------------------- #
# entry point
# --------------------------------------------------------------------------- #

def _prepare(x, src0, dst0, src1, dst1, src2, dst2, n1, n2, n3,
             Wl0, Wr0, b0, Wl1, Wr1, b1, Wl2, Wr2, b2):
    import ml_dtypes
    BF = ml_dtypes.bfloat16
    x16 = np.asarray(x, np.float32).astype(BF)
    src0, dst0 = np.asarray(src0, np.int64), np.asarray(dst0, np.int64)
    src1, dst1 = np.asarray(src1, np.int64), np.asarray(dst1, np.int64)
    src2, dst2 = np.asarray(src2, np.int64), np.asarray(dst2, np.int64)
    assert (int(n1), int(n2), int(n3)) == (N1, N2, N3)

    p0 = _plan_l0(src0, dst0, N1)
    row2 = _owner(np.arange(N2V)) * SP1 + _lpos(np.arange(N2V))
    pl1 = _plan_src_layer(src1, dst1, N2V, p0["row1"], SP0, S0, S1, G1T,
                          rs_chunks=7)
    pl2 = _plan_src_layer(src2, dst2, N3, row2, SP1, S1, S2, G2T,
                          rs_chunks=1)

    meta = dict(
        in_c=x16.shape[1], hid=Wl0.shape[1], out_c=Wl2.shape[1],
        D0=p0["D"], pl1=pl1, pl2=pl2, NB1=pl1["NB"], NB2=pl2["NB"],
    )
    iot = np.tile(np.arange(WIN * P, dtype=np.float32)[None, :], (P, 1))
    in_maps = []
    for k in range(NC):
        xs, xr, rcp, msk = _l0_tables(p0, k, x16)
        m = dict(
            xs0=xs, xr0=xr, rcp0=np.ascontiguousarray(rcp),
            msk0=msk.astype(BF), iot=iot,
            wl0=np.asarray(Wl0, np.float32).astype(BF),
            wr0=np.asarray(Wr0, np.float32).astype(BF),
            b0=np.asarray(b0, np.float32).astype(BF),
            wl1=np.asarray(Wl1, np.float32).astype(BF),
            wr1=np.asarray(Wr1, np.float32).astype(BF),
            b1=np.asarray(b1, np.float32).astype(BF),
            wl2=np.asarray(Wl2, np.float32).astype(BF),
            wr2=np.asarray(Wr2, np.float32).astype(BF),
            b2=np.asarray(b2, np.float32).astype(BF),
        )
        for i, pl in ((1, pl1), (2, pl2)):
            m[f"idx{i}"] = _pack_idx(pl["idx"][k])
            m[f"dr{i}"] = np.ascontiguousarray(
                pl["dr"][k].reshape(-1, P).T.astype(np.float32))
            m[f"ridx{i}"] = _pack_idx(pl["ridx"][k])
            m[f"rcp{i}"] = np.ascontiguousarray(
                pl["rcp"][k].reshape(-1, P).T)
        in_maps.append(m)
    return (p0, pl1, pl2), meta, in_maps


def _assemble(outs):
    t = np.arange(N3)
    full = np.stack(outs)  # [NC, S2, out_c]
    return np.ascontiguousarray(full[_owner(t), _lpos(t)])


def kernel(**inputs) -> np.ndarray:
    from concourse.bass_utils import run_bass_kernel_spmd

    _, meta, in_maps = _prepare(**inputs)
    nc = _build_nc(meta)
    res = run_bass_kernel_spmd(nc, in_maps, core_ids=list(range(NC)))
    return _assemble([res.results[k]["out"] for k in range(NC)])
